# revision 11
# baseline (speedup 1.0000x reference)
"""CDiceLoss Trainium2 kernel.

Shards B*HW over 8 cores (each core = one (batch, half-of-HW) slice).
Per core the bass/Tile kernel computes, over its [20, 131072] slice:
  - G     [20,20] gram (sum_hw x_i x_j)  per 6-channel-group diag blocks
  - sum_x [20]    (ones-column of the gram)
  - sabs  = sum |x + y - 1|    ( = 2*sum(x*y) - sum x - sum y + n )
  - bce   = sum ln|x + y - 1|  ( = sum y*ln(x) + (1-y)*ln(1-x) )
sum_y is an exact host-side integer count; the host combines the tiny
per-core stats into (loss, loss1, loss2, loss3).
"""

import os
from contextlib import ExitStack

import numpy as np
import ml_dtypes

import concourse.bass as bass
import concourse.bacc as bacc
import concourse.tile as tile
from concourse import mybir
from concourse.bass_utils import run_bass_kernel_spmd

# ---------------- problem geometry (hardcoded) ----------------
B, C, H, W = 4, 20, 512, 512
HW = H * W                  # 262144
KNOWN = 16
SMOOTH = 1.0
NCORES = 8
HWH = HW // 2               # 131072 positions per core

# X-tile geometry: rows = (g c) with NG channel-groups, block length FB.
NG, FB = 6, 4096            # main tiles [121, 4096], cover NG*FB = 24576 pos
NMAIN = 5                   # 5 main tiles = 122880 positions
TG, TFB = 4, 2048           # tail tile [81, 2048], covers 8192 positions
assert NMAIN * NG * FB + TG * TFB == HWH
NTILES = NMAIN + 1

FP32 = mybir.dt.float32
BF16 = mybir.dt.bfloat16
I32 = mybir.dt.int32
AX = mybir.AxisListType
OP = mybir.AluOpType
AF = mybir.ActivationFunctionType

_CACHE = {}


def _build():
    """Build (and cache) the per-core bass program."""
    if "nc" in _CACHE:
        return _CACHE["nc"]

    nc = bacc.Bacc(
        "TRN2", target_bir_lowering=False, debug=False, num_devices=NCORES
    )

    x_d = nc.dram_tensor("x", [C, HWH], FP32, kind="ExternalInput").ap()
    y_d = nc.dram_tensor("y", [C, HWH], I32, kind="ExternalInput").ap()
    id_d = nc.dram_tensor("ident", [128, 128], BF16, kind="ExternalInput").ap()
    on_d = nc.dram_tensor("ones", [48, FB], BF16, kind="ExternalInput").ap()

    g_d = nc.dram_tensor("g_out", [128, 128], FP32, kind="ExternalOutput").ap()
    g2_d = nc.dram_tensor("g2_out", [81, 81], FP32, kind="ExternalOutput").ap()
    st_d = nc.dram_tensor("st_out", [128, 3 * NTILES], FP32, kind="ExternalOutput").ap()

    with tile.TileContext(nc) as tc, ExitStack() as ctx:
        sing = ctx.enter_context(tc.tile_pool(name="sing", bufs=1))
        xpool = ctx.enter_context(tc.tile_pool(name="xpool", bufs=2))
        ypool = ctx.enter_context(tc.tile_pool(name="ypool", bufs=2))
        epool = ctx.enter_context(tc.tile_pool(name="epool", bufs=2))
        spool = ctx.enter_context(tc.tile_pool(name="spool", bufs=3))
        pst_pool = ctx.enter_context(tc.tile_pool(name="pst", bufs=3, space="PSUM"))
        gp_pool = ctx.enter_context(tc.tile_pool(name="gp", bufs=1, space="PSUM"))

        ident = sing.tile([128, 128], BF16)
        nc.sync.dma_start(out=ident[:, :], in_=id_d)
        ones_sb = sing.tile([48, FB], BF16)
        nc.sync.dma_start(out=ones_sb[:, :], in_=on_d)

        # stats accumulator columns: [num | sxy | bce] per tile
        stats = sing.tile([128, 3 * NTILES], FP32)
        nc.vector.memset(stats[:, :], 0.0)
        dummy = sing.tile([128, 1], BF16)

        g_ps = gp_pool.tile([128, 121], FP32)
        g2_ps = gp_pool.tile([128, 81], FP32)

        # Safety: make sure all 128x128 PE weight cells hold finite values
        # before K<128 stationary loads leave stale rows in the array.
        warm = pst_pool.tile([128, 128], BF16)
        nc.tensor.transpose(out=warm[:, :], in_=ident[:, :], identity=ident[:, :])

        for t in range(NTILES):
            if t < NMAIN:
                ng, fb = NG, FB
            else:
                ng, fb = TG, TFB
            rows = ng * C            # 120 or 80
            rp1 = rows + 1           # + ones row
            off = t * NG * FB        # position offset of this tile
            nsub = fb // 128         # 32 or 16 sub-slabs

            # ---- X load: f32 DRAM -> bf16 SBUF (SWDGE cast), one DMA per
            # channel-group so every AP stays plain 2-D.
            xt = xpool.tile([128, FB], BF16, tag="xt")
            for g in range(ng):
                nc.gpsimd.dma_start(
                    out=xt[g * C : (g + 1) * C, 0:fb],
                    in_=x_d[:, off + g * fb : off + (g + 1) * fb],
                )
            nc.sync.dma_start(out=xt[rows:128, 0:fb], in_=ones_sb[0 : 128 - rows, 0:fb])

            # ---- Y load: int32, same row layout (HWDGE)
            yt = ypool.tile([120, FB], I32, tag="yt")
            for g in range(ng):
                nc.sync.dma_start(
                    out=yt[g * C : (g + 1) * C, 0:fb],
                    in_=y_d[:, off + g * fb : off + (g + 1) * fb],
                )

            # ---- ym1 = y - 1 in bf16 (GPSIMD; also the int->float cast)
            ym1 = ypool.tile([120, FB], BF16, tag="ym1")
            nc.gpsimd.tensor_scalar(
                out=ym1[0:rows, 0:fb],
                in0=yt[0:rows, 0:fb],
                scalar1=1.0,
                scalar2=None,
                op0=OP.subtract,
            )

            # ---- s1m1 = x + (y-1)  (plain DVE tensor_tensor, bf16 2x)
            s1m1 = epool.tile([120, FB], BF16, tag="s1m1")
            nc.vector.tensor_tensor(
                s1m1[0:rows, 0:fb],
                xt[0:rows, 0:fb],
                ym1[0:rows, 0:fb],
                OP.add,
            )

            # ---- |x+y-1| with free accumulate -> sum|x+y-1| (ACT)
            absz = epool.tile([120, FB], BF16, tag="absz")
            nc.scalar.activation(
                out=absz[0:rows, 0:fb],
                in_=s1m1[0:rows, 0:fb],
                func=AF.Abs,
                accum_out=stats[0:rows, 3 * t : 3 * t + 1],
            )

            # ---- bce partial: sum ln|x+y-1| (ACT with free accumulate)
            lnz = epool.tile([120, FB], BF16, tag="lnz")
            nc.scalar.activation(
                out=lnz[0:rows, 0:fb],
                in_=absz[0:rows, 0:fb],
                func=AF.Ln,
                accum_out=stats[0:rows, 3 * t + 2 : 3 * t + 3],
            )

            # ---- gram path: per 128-column sub-slab transpose + matmul
            for jg in range(nsub // 4):
                pst = pst_pool.tile([128, 512], BF16, tag="pst")
                for jj in range(4):
                    j = jg * 4 + jj
                    nc.tensor.transpose(
                        out=pst[:, jj * 128 : (jj + 1) * 128],
                        in_=xt[:, j * 128 : (j + 1) * 128],
                        identity=ident[:, :],
                    )
                stsb = spool.tile([128, 512], BF16, tag="stsb")
                nc.vector.tensor_copy(out=stsb[:, 0:512], in_=pst[:, 0:512])
                for jj in range(4):
                    cof = jj * 128
                    if t < NMAIN:
                        nc.tensor.matmul(
                            out=g_ps[:, :],
                            lhsT=stsb[:, cof : cof + 128],
                            rhs=stsb[:, cof : cof + 121],
                            start=(t == 0 and jg == 0 and jj == 0),
                            stop=(t == NMAIN - 1 and jg == nsub // 4 - 1 and jj == 3),
                            skip_group_check=True,
                        )
                    else:
                        nc.tensor.matmul(
                            out=g2_ps[:, :],
                            lhsT=stsb[:, cof : cof + 128],
                            rhs=stsb[:, cof : cof + 81],
                            start=(jg == 0 and jj == 0),
                            stop=(jg == nsub // 4 - 1 and jj == 3),
                            skip_group_check=True,
                        )

        # ---- write results out
        g_sb = sing.tile([128, 128], FP32)
        nc.vector.tensor_copy(out=g_sb[:, 0:121], in_=g_ps[:, :])
        nc.vector.memset(g_sb[:, 121:128], 0.0)
        g2_sb = sing.tile([81, 81], FP32)
        nc.scalar.copy(out=g2_sb[:, :], in_=g2_ps[0:81, :])
        nc.sync.dma_start(out=g_d, in_=g_sb[:, :])
        nc.sync.dma_start(out=g2_d, in_=g2_sb[:, :])
        nc.sync.dma_start(out=st_d, in_=stats[:, :])

    nc.compile()
    _CACHE["nc"] = nc
    return nc


def _run(logit, label_lst, trace=False):
    nc = _build()
    X = np.asarray(logit, dtype=np.float32).reshape(B, C, HW)
    Y = np.asarray(label_lst, dtype=np.int32).reshape(B, C, HW)
    ident = np.eye(128, dtype=ml_dtypes.bfloat16)
    ones = np.ones((48, FB), dtype=ml_dtypes.bfloat16)

    in_maps = []
    for k in range(NCORES):
        b, half = k // 2, k % 2
        in_maps.append(
            {
                "x": np.ascontiguousarray(X[b, :, half * HWH : (half + 1) * HWH]),
                "y": np.ascontiguousarray(Y[b, :, half * HWH : (half + 1) * HWH]),
                "ident": ident,
                "ones": ones,
            }
        )
    res = run_bass_kernel_spmd(
        nc, in_maps, list(range(NCORES)), trace=trace
    )
    return res


def _combine(results, sum_y):
    """Host-side tiny combine of per-core stats."""
    G = np.zeros((B, C, C), dtype=np.float64)
    sum_x = np.zeros((B, C), dtype=np.float64)
    sabs_r = np.zeros((B, C), dtype=np.float64)
    bce_r = np.zeros((B, C), dtype=np.float64)

    for k in range(NCORES):
        b = k // 2
        r = results[k]
        g = r["g_out"].astype(np.float64)
        g2 = r["g2_out"].astype(np.float64)
        st = r["st_out"].astype(np.float64)
        for gi in range(NG):
            sl = slice(gi * C, gi * C + C)
            G[b] += g[sl, sl]
            sum_x[b] += g[sl, 120]
        for gi in range(TG):
            sl = slice(gi * C, gi * C + C)
            G[b] += g2[sl, sl]
            sum_x[b] += g2[sl, 80]
        for t in range(NTILES):
            ng = NG if t < NMAIN else TG
            cols = st[: ng * C, 3 * t : 3 * t + 3].reshape(ng, C, 3)
            sabs_r[b] += cols[:, :, 0].sum(axis=0)
            bce_r[b] += cols[:, :, 2].sum(axis=0)

    # |x+y-1| = 2xy - x - y + 1  =>  sum(xy) = (sabs + sum_x + sum_y - HW)/2
    num = 0.5 * (sabs_r + sum_x + sum_y - HW)
    s = np.einsum("bii->bi", G)              # sum x^2

    # loss1
    numk = num[:, :KNOWN] + SMOOTH
    denk = s[:, :KNOWN] + sum_y[:, :KNOWN] + SMOOTH
    dice = np.mean(1.0 - numk / denk, axis=0)
    bce = -bce_r[:, :KNOWN].sum(axis=0) / (B * HW)
    loss1 = (dice + bce).sum() / KNOWN

    # loss2
    m = sum_x[:, KNOWN:].sum(axis=0) / (B * HW)
    loss2 = np.sum(-np.log(np.clip(m * 50.0, 1e-300, 1.0))) / (C - KNOWN)

    # loss3
    ratio = (G + SMOOTH) / (s[:, :, None] + s[:, None, :] + SMOOTH)
    M = ratio.mean(axis=0)
    loss3 = (M.sum() - np.trace(M)) / (C * (C - 1))

    loss = (loss1 + loss2 + loss3) * 0.1
    f = np.float32
    return f(loss), f(loss1), f(loss2), f(loss3)


def kernel(logit, label_lst, class_lst=None, **_):
    Y = np.asarray(label_lst, dtype=np.int64).reshape(B, C, HW)
    sum_y = Y.sum(axis=2).astype(np.float64)
    res = _run(logit, label_lst, trace=bool(os.environ.get("CDICE_TRACE")))
    out = _combine(res.results, sum_y)
    if os.environ.get("CDICE_TRACE"):
        kernel.last_result = res
    return out


# revision 12
# speedup vs baseline: 2.7578x; 2.7578x over previous
"""CDiceLoss Trainium2 kernel.

Shards B*HW over 8 cores (each core = one (batch, half-of-HW) slice).
Per core the bass/Tile kernel computes, over its [20, 131072] slice:
  - G     [20,20] gram (sum_hw x_i x_j)  per 6-channel-group diag blocks
  - sum_x [20]    (ones-column of the gram)
  - sabs  = sum |x + y - 1|    ( = 2*sum(x*y) - sum x - sum y + n )
  - bce   = sum ln|x + y - 1|  ( = sum y*ln(x) + (1-y)*ln(1-x) )
sum_y is an exact host-side integer count; the host combines the tiny
per-core stats into (loss, loss1, loss2, loss3).
"""

import os
from contextlib import ExitStack

import numpy as np
import ml_dtypes

import concourse.bass as bass
import concourse.bacc as bacc
import concourse.tile as tile
from concourse import mybir
from concourse.bass_utils import run_bass_kernel_spmd

# ---------------- problem geometry (hardcoded) ----------------
B, C, H, W = 4, 20, 512, 512
HW = H * W                  # 262144
KNOWN = 16
SMOOTH = 1.0
NCORES = 8
HWH = HW // 2               # 131072 positions per core

# X-tile geometry: rows = (g c) with NG channel-groups, block length FB.
NG, FB = 6, 4096            # main tiles [121, 4096], cover NG*FB = 24576 pos
NMAIN = 5                   # 5 main tiles = 122880 positions
TG, TFB = 4, 2048           # tail tile [81, 2048], covers 8192 positions
assert NMAIN * NG * FB + TG * TFB == HWH
NTILES = NMAIN + 1

FP32 = mybir.dt.float32
BF16 = mybir.dt.bfloat16
I32 = mybir.dt.int32
AX = mybir.AxisListType
OP = mybir.AluOpType
AF = mybir.ActivationFunctionType

_CACHE = {}


def _build():
    """Build (and cache) the per-core bass program."""
    if "nc" in _CACHE:
        return _CACHE["nc"]

    nc = bacc.Bacc(
        "TRN2", target_bir_lowering=False, debug=False, num_devices=NCORES
    )

    x_d = nc.dram_tensor("x", [C, HWH], FP32, kind="ExternalInput").ap()
    y_d = nc.dram_tensor("y", [C, HWH], I32, kind="ExternalInput").ap()
    id_d = nc.dram_tensor("ident", [128, 128], BF16, kind="ExternalInput").ap()
    on_d = nc.dram_tensor("ones", [48, FB], BF16, kind="ExternalInput").ap()

    g_d = nc.dram_tensor("g_out", [128, 128], FP32, kind="ExternalOutput").ap()
    g2_d = nc.dram_tensor("g2_out", [81, 81], FP32, kind="ExternalOutput").ap()
    st_d = nc.dram_tensor("st_out", [128, 3 * NTILES], FP32, kind="ExternalOutput").ap()

    with tile.TileContext(nc) as tc, ExitStack() as ctx:
        sing = ctx.enter_context(tc.tile_pool(name="sing", bufs=1))
        xpool = ctx.enter_context(tc.tile_pool(name="xpool", bufs=2))
        ypool = ctx.enter_context(tc.tile_pool(name="ypool", bufs=2))
        epool = ctx.enter_context(tc.tile_pool(name="epool", bufs=2))
        spool = ctx.enter_context(tc.tile_pool(name="spool", bufs=3))
        pst_pool = ctx.enter_context(tc.tile_pool(name="pst", bufs=3, space="PSUM"))
        gp_pool = ctx.enter_context(tc.tile_pool(name="gp", bufs=1, space="PSUM"))

        ident = sing.tile([128, 128], BF16)
        nc.sync.dma_start(out=ident[:, :], in_=id_d)
        ones_sb = sing.tile([48, FB], BF16)
        nc.sync.dma_start(out=ones_sb[:, :], in_=on_d)

        # stats accumulator columns: [num | sxy | bce] per tile
        stats = sing.tile([128, 3 * NTILES], FP32)
        nc.vector.memset(stats[:, :], 0.0)
        mone = sing.tile([128, 1], FP32)
        nc.vector.memset(mone[:, :], -1.0)

        g_ps = gp_pool.tile([128, 121], FP32)
        g2_ps = gp_pool.tile([128, 81], FP32)

        # Safety: make sure all 128x128 PE weight cells hold finite values
        # before K<128 stationary loads leave stale rows in the array.
        warm = pst_pool.tile([128, 128], BF16)
        nc.tensor.transpose(out=warm[:, :], in_=ident[:, :], identity=ident[:, :])

        for t in range(NTILES):
            if t < NMAIN:
                ng, fb = NG, FB
            else:
                ng, fb = TG, TFB
            rows = ng * C            # 120 or 80
            rp1 = rows + 1           # + ones row
            off = t * NG * FB        # position offset of this tile
            nsub = fb // 128         # 32 or 16 sub-slabs

            # ---- X load: f32 DRAM -> bf16 SBUF (SWDGE cast), one DMA per
            # channel-group so every AP stays plain 2-D.
            xt = xpool.tile([128, FB], BF16, tag="xt")
            for g in range(ng):
                nc.gpsimd.dma_start(
                    out=xt[g * C : (g + 1) * C, 0:fb],
                    in_=x_d[:, off + g * fb : off + (g + 1) * fb],
                )
            nc.sync.dma_start(out=xt[rows:128, 0:fb], in_=ones_sb[0 : 128 - rows, 0:fb])

            # ---- Y load: int32, same row layout (HWDGE)
            yt = ypool.tile([120, FB], I32, tag="yt")
            for g in range(ng):
                nc.sync.dma_start(
                    out=yt[g * C : (g + 1) * C, 0:fb],
                    in_=y_d[:, off + g * fb : off + (g + 1) * fb],
                )

            # ---- s1 = x + y (mixed dtype TT: bf16 + int32 -> bf16)
            s1m1 = epool.tile([120, FB], BF16, tag="s1m1")
            nc.vector.tensor_tensor(
                s1m1[0:rows, 0:fb],
                xt[0:rows, 0:fb],
                yt[0:rows, 0:fb],
                OP.add,
            )

            # ---- |x+y-1| with free accumulate -> sum|x+y-1| (ACT, bias=-1)
            absz = epool.tile([120, FB], BF16, tag="absz")
            nc.scalar.activation(
                out=absz[0:rows, 0:fb],
                in_=s1m1[0:rows, 0:fb],
                func=AF.Abs,
                bias=mone[0:rows, :],
                accum_out=stats[0:rows, 3 * t : 3 * t + 1],
            )

            # ---- bce partial: sum ln|x+y-1| (ACT with free accumulate)
            lnz = epool.tile([120, FB], BF16, tag="lnz")
            nc.scalar.activation(
                out=lnz[0:rows, 0:fb],
                in_=absz[0:rows, 0:fb],
                func=AF.Ln,
                accum_out=stats[0:rows, 3 * t + 2 : 3 * t + 3],
            )

            # ---- gram path: per 128-column sub-slab transpose + matmul
            for jg in range(nsub // 4):
                pst = pst_pool.tile([128, 512], BF16, tag="pst")
                for jj in range(4):
                    j = jg * 4 + jj
                    nc.tensor.transpose(
                        out=pst[:, jj * 128 : (jj + 1) * 128],
                        in_=xt[:, j * 128 : (j + 1) * 128],
                        identity=ident[:, :],
                    )
                stsb = spool.tile([128, 512], BF16, tag="stsb")
                nc.vector.tensor_copy(out=stsb[:, 0:512], in_=pst[:, 0:512])
                for jj in range(4):
                    cof = jj * 128
                    if t < NMAIN:
                        nc.tensor.matmul(
                            out=g_ps[:, :],
                            lhsT=stsb[:, cof : cof + 128],
                            rhs=stsb[:, cof : cof + 121],
                            start=(t == 0 and jg == 0 and jj == 0),
                            stop=(t == NMAIN - 1 and jg == nsub // 4 - 1 and jj == 3),
                            skip_group_check=True,
                        )
                    else:
                        nc.tensor.matmul(
                            out=g2_ps[:, :],
                            lhsT=stsb[:, cof : cof + 128],
                            rhs=stsb[:, cof : cof + 81],
                            start=(jg == 0 and jj == 0),
                            stop=(jg == nsub // 4 - 1 and jj == 3),
                            skip_group_check=True,
                        )

        # ---- write results out
        g_sb = sing.tile([128, 128], FP32)
        nc.vector.tensor_copy(out=g_sb[:, 0:121], in_=g_ps[:, :])
        nc.vector.memset(g_sb[:, 121:128], 0.0)
        g2_sb = sing.tile([81, 81], FP32)
        nc.scalar.copy(out=g2_sb[:, :], in_=g2_ps[0:81, :])
        nc.sync.dma_start(out=g_d, in_=g_sb[:, :])
        nc.sync.dma_start(out=g2_d, in_=g2_sb[:, :])
        nc.sync.dma_start(out=st_d, in_=stats[:, :])

    nc.compile()
    _CACHE["nc"] = nc
    return nc


def _run(logit, label_lst, trace=False):
    nc = _build()
    X = np.asarray(logit, dtype=np.float32).reshape(B, C, HW)
    Y = np.asarray(label_lst, dtype=np.int32).reshape(B, C, HW)
    ident = np.eye(128, dtype=ml_dtypes.bfloat16)
    ones = np.ones((48, FB), dtype=ml_dtypes.bfloat16)

    in_maps = []
    for k in range(NCORES):
        b, half = k // 2, k % 2
        in_maps.append(
            {
                "x": np.ascontiguousarray(X[b, :, half * HWH : (half + 1) * HWH]),
                "y": np.ascontiguousarray(Y[b, :, half * HWH : (half + 1) * HWH]),
                "ident": ident,
                "ones": ones,
            }
        )
    res = run_bass_kernel_spmd(
        nc, in_maps, list(range(NCORES)), trace=trace
    )
    return res


def _combine(results, sum_y):
    """Host-side tiny combine of per-core stats."""
    G = np.zeros((B, C, C), dtype=np.float64)
    sum_x = np.zeros((B, C), dtype=np.float64)
    sabs_r = np.zeros((B, C), dtype=np.float64)
    bce_r = np.zeros((B, C), dtype=np.float64)

    for k in range(NCORES):
        b = k // 2
        r = results[k]
        g = r["g_out"].astype(np.float64)
        g2 = r["g2_out"].astype(np.float64)
        st = r["st_out"].astype(np.float64)
        for gi in range(NG):
            sl = slice(gi * C, gi * C + C)
            G[b] += g[sl, sl]
            sum_x[b] += g[sl, 120]
        for gi in range(TG):
            sl = slice(gi * C, gi * C + C)
            G[b] += g2[sl, sl]
            sum_x[b] += g2[sl, 80]
        for t in range(NTILES):
            ng = NG if t < NMAIN else TG
            cols = st[: ng * C, 3 * t : 3 * t + 3].reshape(ng, C, 3)
            sabs_r[b] += cols[:, :, 0].sum(axis=0)
            bce_r[b] += cols[:, :, 2].sum(axis=0)

    # |x+y-1| = 2xy - x - y + 1  =>  sum(xy) = (sabs + sum_x + sum_y - HW)/2
    num = 0.5 * (sabs_r + sum_x + sum_y - HW)
    s = np.einsum("bii->bi", G)              # sum x^2

    # loss1
    numk = num[:, :KNOWN] + SMOOTH
    denk = s[:, :KNOWN] + sum_y[:, :KNOWN] + SMOOTH
    dice = np.mean(1.0 - numk / denk, axis=0)
    bce = -bce_r[:, :KNOWN].sum(axis=0) / (B * HW)
    loss1 = (dice + bce).sum() / KNOWN

    # loss2
    m = sum_x[:, KNOWN:].sum(axis=0) / (B * HW)
    loss2 = np.sum(-np.log(np.clip(m * 50.0, 1e-300, 1.0))) / (C - KNOWN)

    # loss3
    ratio = (G + SMOOTH) / (s[:, :, None] + s[:, None, :] + SMOOTH)
    M = ratio.mean(axis=0)
    loss3 = (M.sum() - np.trace(M)) / (C * (C - 1))

    loss = (loss1 + loss2 + loss3) * 0.1
    f = np.float32
    return f(loss), f(loss1), f(loss2), f(loss3)


def kernel(logit, label_lst, class_lst=None, **_):
    Y = np.asarray(label_lst, dtype=np.int64).reshape(B, C, HW)
    sum_y = Y.sum(axis=2).astype(np.float64)
    res = _run(logit, label_lst, trace=bool(os.environ.get("CDICE_TRACE")))
    out = _combine(res.results, sum_y)
    if os.environ.get("CDICE_TRACE"):
        kernel.last_result = res
    return out


# revision 14
# speedup vs baseline: 2.8517x; 1.0340x over previous
"""CDiceLoss Trainium2 kernel.

Shards B*HW over 8 cores (each core = one (batch, half-of-HW) slice).
Per core the bass/Tile kernel computes, over its [20, 131072] slice:
  - G     [20,20] gram (sum_hw x_i x_j)  per 6-channel-group diag blocks
  - sum_x [20]    (ones-column of the gram)
  - sabs  = sum |x + y - 1|    ( = 2*sum(x*y) - sum x - sum y + n )
  - bce   = sum ln|x + y - 1|  ( = sum y*ln(x) + (1-y)*ln(1-x) )
sum_y is an exact host-side integer count; the host combines the tiny
per-core stats into (loss, loss1, loss2, loss3).
"""

import os
from contextlib import ExitStack

import numpy as np
import ml_dtypes

import concourse.bass as bass
import concourse.bacc as bacc
import concourse.tile as tile
from concourse import mybir
from concourse.bass_utils import run_bass_kernel_spmd

# ---------------- problem geometry (hardcoded) ----------------
B, C, H, W = 4, 20, 512, 512
HW = H * W                  # 262144
KNOWN = 16
SMOOTH = 1.0
NCORES = 8
HWH = HW // 2               # 131072 positions per core

# X-tile geometry: rows = (g c) with NG channel-groups, block length FB.
NG, FB = 6, 4096            # main tiles [121, 4096], cover NG*FB = 24576 pos
NMAIN = 5                   # 5 main tiles = 122880 positions
TG, TFB = 4, 2048           # tail tile [81, 2048], covers 8192 positions
assert NMAIN * NG * FB + TG * TFB == HWH
NTILES = NMAIN + 1

FP32 = mybir.dt.float32
BF16 = mybir.dt.bfloat16
I32 = mybir.dt.int32
I8 = mybir.dt.int8
AX = mybir.AxisListType
OP = mybir.AluOpType
AF = mybir.ActivationFunctionType

_CACHE = {}


def _build():
    """Build (and cache) the per-core bass program."""
    if "nc" in _CACHE:
        return _CACHE["nc"]

    nc = bacc.Bacc(
        "TRN2", target_bir_lowering=False, debug=False, num_devices=NCORES
    )

    x_d = nc.dram_tensor("x", [C, HWH], FP32, kind="ExternalInput").ap()
    y_d = nc.dram_tensor("y", [C, HWH], I8, kind="ExternalInput").ap()
    id_d = nc.dram_tensor("ident", [128, 128], BF16, kind="ExternalInput").ap()
    on_d = nc.dram_tensor("ones", [48, FB], BF16, kind="ExternalInput").ap()

    g_d = nc.dram_tensor("g_out", [128, 128], FP32, kind="ExternalOutput").ap()
    g2_d = nc.dram_tensor("g2_out", [81, 81], FP32, kind="ExternalOutput").ap()
    st_d = nc.dram_tensor("st_out", [128, 3 * NTILES], FP32, kind="ExternalOutput").ap()

    with tile.TileContext(nc) as tc, ExitStack() as ctx:
        sing = ctx.enter_context(tc.tile_pool(name="sing", bufs=1))
        xpool = ctx.enter_context(tc.tile_pool(name="xpool", bufs=2))
        xfpool = ctx.enter_context(tc.tile_pool(name="xfpool", bufs=2))
        ypool = ctx.enter_context(tc.tile_pool(name="ypool", bufs=2))
        epool = ctx.enter_context(tc.tile_pool(name="epool", bufs=2))
        spool = ctx.enter_context(tc.tile_pool(name="spool", bufs=3))
        pst_pool = ctx.enter_context(tc.tile_pool(name="pst", bufs=3, space="PSUM"))
        gp_pool = ctx.enter_context(tc.tile_pool(name="gp", bufs=1, space="PSUM"))

        ident = sing.tile([128, 128], BF16)
        nc.sync.dma_start(out=ident[:, :], in_=id_d)
        ones_sb = sing.tile([48, FB], BF16)
        nc.sync.dma_start(out=ones_sb[:, :], in_=on_d)

        # stats accumulator columns: [num | sxy | bce] per tile
        stats = sing.tile([128, 3 * NTILES], FP32)
        nc.vector.memset(stats[:, :], 0.0)
        mone = sing.tile([128, 1], FP32)
        nc.vector.memset(mone[:, :], -1.0)

        g_ps = gp_pool.tile([128, 121], FP32)
        g2_ps = gp_pool.tile([128, 81], FP32)

        # Safety: make sure all 128x128 PE weight cells hold finite values
        # before K<128 stationary loads leave stale rows in the array.
        warm = pst_pool.tile([128, 128], BF16)
        nc.tensor.transpose(out=warm[:, :], in_=ident[:, :], identity=ident[:, :])

        for t in range(NTILES):
            if t < NMAIN:
                ng, fb = NG, FB
            else:
                ng, fb = TG, TFB
            rows = ng * C            # 120 or 80
            rp1 = rows + 1           # + ones row
            off = t * NG * FB        # position offset of this tile
            nsub = fb // 128         # 32 or 16 sub-slabs

            # ---- X load: f32 via HWDGE (balanced across SDMA engines),
            # then DVE cast f32 -> bf16.
            xf = xfpool.tile([120, FB], FP32, tag="xf")
            for g in range(ng):
                nc.sync.dma_start(
                    out=xf[g * C : (g + 1) * C, 0:fb],
                    in_=x_d[:, off + g * fb : off + (g + 1) * fb],
                )
            xt = xpool.tile([128, FB], BF16, tag="xt")
            nc.vector.tensor_copy(out=xt[0:rows, 0:fb], in_=xf[0:rows, 0:fb])
            nc.sync.dma_start(out=xt[rows:128, 0:fb], in_=ones_sb[0 : 128 - rows, 0:fb])

            # ---- Y load: int8 DRAM -> bf16 SBUF (SWDGE cast DMA, tiny bytes)
            yt = ypool.tile([120, FB], BF16, tag="yt")
            for g in range(ng):
                nc.gpsimd.dma_start(
                    out=yt[g * C : (g + 1) * C, 0:fb],
                    in_=y_d[:, off + g * fb : off + (g + 1) * fb],
                )

            # ---- s1 = x + y (bf16 TT, 2x mode)
            s1m1 = epool.tile([120, FB], BF16, tag="s1m1")
            nc.vector.tensor_tensor(
                s1m1[0:rows, 0:fb],
                xt[0:rows, 0:fb],
                yt[0:rows, 0:fb],
                OP.add,
            )

            # ---- |x+y-1| with free accumulate -> sum|x+y-1| (ACT, bias=-1)
            absz = epool.tile([120, FB], BF16, tag="absz")
            nc.scalar.activation(
                out=absz[0:rows, 0:fb],
                in_=s1m1[0:rows, 0:fb],
                func=AF.Abs,
                bias=mone[0:rows, :],
                accum_out=stats[0:rows, 3 * t : 3 * t + 1],
            )

            # ---- bce partial: sum ln|x+y-1| (ACT with free accumulate)
            lnz = epool.tile([120, FB], BF16, tag="lnz")
            nc.scalar.activation(
                out=lnz[0:rows, 0:fb],
                in_=absz[0:rows, 0:fb],
                func=AF.Ln,
                accum_out=stats[0:rows, 3 * t + 2 : 3 * t + 3],
            )

            # ---- gram path: per 128-column sub-slab transpose + matmul
            for jg in range(nsub // 4):
                pst = pst_pool.tile([128, 512], BF16, tag="pst")
                for jj in range(4):
                    j = jg * 4 + jj
                    nc.tensor.transpose(
                        out=pst[:, jj * 128 : (jj + 1) * 128],
                        in_=xt[:, j * 128 : (j + 1) * 128],
                        identity=ident[:, :],
                    )
                stsb = spool.tile([128, 512], BF16, tag="stsb")
                nc.vector.tensor_copy(out=stsb[:, 0:512], in_=pst[:, 0:512])
                for jj in range(4):
                    cof = jj * 128
                    if t < NMAIN:
                        nc.tensor.matmul(
                            out=g_ps[:, :],
                            lhsT=stsb[:, cof : cof + 128],
                            rhs=stsb[:, cof : cof + 121],
                            start=(t == 0 and jg == 0 and jj == 0),
                            stop=(t == NMAIN - 1 and jg == nsub // 4 - 1 and jj == 3),
                            skip_group_check=True,
                        )
                    else:
                        nc.tensor.matmul(
                            out=g2_ps[:, :],
                            lhsT=stsb[:, cof : cof + 128],
                            rhs=stsb[:, cof : cof + 81],
                            start=(jg == 0 and jj == 0),
                            stop=(jg == nsub // 4 - 1 and jj == 3),
                            skip_group_check=True,
                        )

        # ---- write results out
        g_sb = sing.tile([128, 128], FP32)
        nc.vector.tensor_copy(out=g_sb[:, 0:121], in_=g_ps[:, :])
        nc.vector.memset(g_sb[:, 121:128], 0.0)
        g2_sb = sing.tile([81, 81], FP32)
        nc.scalar.copy(out=g2_sb[:, :], in_=g2_ps[0:81, :])
        nc.sync.dma_start(out=g_d, in_=g_sb[:, :])
        nc.sync.dma_start(out=g2_d, in_=g2_sb[:, :])
        nc.sync.dma_start(out=st_d, in_=stats[:, :])

    nc.compile()
    _CACHE["nc"] = nc
    return nc


def _run(logit, label_lst, trace=False):
    nc = _build()
    X = np.asarray(logit, dtype=np.float32).reshape(B, C, HW)
    Y = np.asarray(label_lst).reshape(B, C, HW).astype(np.int8)
    ident = np.eye(128, dtype=ml_dtypes.bfloat16)
    ones = np.ones((48, FB), dtype=ml_dtypes.bfloat16)

    in_maps = []
    for k in range(NCORES):
        b, half = k // 2, k % 2
        in_maps.append(
            {
                "x": np.ascontiguousarray(X[b, :, half * HWH : (half + 1) * HWH]),
                "y": np.ascontiguousarray(Y[b, :, half * HWH : (half + 1) * HWH]),
                "ident": ident,
                "ones": ones,
            }
        )
    res = run_bass_kernel_spmd(
        nc, in_maps, list(range(NCORES)), trace=trace
    )
    return res


def _combine(results, sum_y):
    """Host-side tiny combine of per-core stats."""
    G = np.zeros((B, C, C), dtype=np.float64)
    sum_x = np.zeros((B, C), dtype=np.float64)
    sabs_r = np.zeros((B, C), dtype=np.float64)
    bce_r = np.zeros((B, C), dtype=np.float64)

    for k in range(NCORES):
        b = k // 2
        r = results[k]
        g = r["g_out"].astype(np.float64)
        g2 = r["g2_out"].astype(np.float64)
        st = r["st_out"].astype(np.float64)
        for gi in range(NG):
            sl = slice(gi * C, gi * C + C)
            G[b] += g[sl, sl]
            sum_x[b] += g[sl, 120]
        for gi in range(TG):
            sl = slice(gi * C, gi * C + C)
            G[b] += g2[sl, sl]
            sum_x[b] += g2[sl, 80]
        for t in range(NTILES):
            ng = NG if t < NMAIN else TG
            cols = st[: ng * C, 3 * t : 3 * t + 3].reshape(ng, C, 3)
            sabs_r[b] += cols[:, :, 0].sum(axis=0)
            bce_r[b] += cols[:, :, 2].sum(axis=0)

    # |x+y-1| = 2xy - x - y + 1  =>  sum(xy) = (sabs + sum_x + sum_y - HW)/2
    num = 0.5 * (sabs_r + sum_x + sum_y - HW)
    s = np.einsum("bii->bi", G)              # sum x^2

    # loss1
    numk = num[:, :KNOWN] + SMOOTH
    denk = s[:, :KNOWN] + sum_y[:, :KNOWN] + SMOOTH
    dice = np.mean(1.0 - numk / denk, axis=0)
    bce = -bce_r[:, :KNOWN].sum(axis=0) / (B * HW)
    loss1 = (dice + bce).sum() / KNOWN

    # loss2
    m = sum_x[:, KNOWN:].sum(axis=0) / (B * HW)
    loss2 = np.sum(-np.log(np.clip(m * 50.0, 1e-300, 1.0))) / (C - KNOWN)

    # loss3
    ratio = (G + SMOOTH) / (s[:, :, None] + s[:, None, :] + SMOOTH)
    M = ratio.mean(axis=0)
    loss3 = (M.sum() - np.trace(M)) / (C * (C - 1))

    loss = (loss1 + loss2 + loss3) * 0.1
    f = np.float32
    return f(loss), f(loss1), f(loss2), f(loss3)


def kernel(logit, label_lst, class_lst=None, **_):
    Y = np.asarray(label_lst, dtype=np.int64).reshape(B, C, HW)
    sum_y = Y.sum(axis=2).astype(np.float64)
    res = _run(logit, label_lst, trace=bool(os.environ.get("CDICE_TRACE")))
    out = _combine(res.results, sum_y)
    if os.environ.get("CDICE_TRACE"):
        kernel.last_result = res
    return out


# revision 15
# speedup vs baseline: 3.7330x; 1.3090x over previous
"""CDiceLoss Trainium2 kernel.

Shards B*HW over 8 cores (core = one (batch, half-of-HW) slice). The host
packs each core's logit slice into a [120, 21888] f32 "slab" (rows =
(group g, channel c), 6 groups per channel, zero-padded tail — x=0,y=0
pads are neutral in every statistic) and labels into the same layout as
int8. Per core the Tile kernel computes:
  - G     [128,121] PSUM gram: diag 20x20 blocks sum to sum_hw x_i x_j,
          ones-column 120 gives sum_x per row
  - sabs  = sum |x + y - 1|    ( = 2*sum(x*y) - sum x - sum y + n )
  - bce   = sum ln|x + y - 1|  ( = sum y*ln(x) + (1-y)*ln(1-x) )
sum_y is an exact host-side integer count; the host combines the tiny
per-core stats into (loss, loss1, loss2, loss3).
"""

import os
from contextlib import ExitStack

import numpy as np
import ml_dtypes

import concourse.bass as bass
import concourse.bacc as bacc
import concourse.tile as tile
from concourse import mybir
from concourse.bass_utils import run_bass_kernel_spmd

# ---------------- problem geometry (hardcoded) ----------------
B, C, H, W = 4, 20, 512, 512
HW = H * W                  # 262144
KNOWN = 16
SMOOTH = 1.0
NCORES = 8
HWH = HW // 2               # 131072 positions per core
NG = 6                      # channel-groups per slab
L = 21888                   # padded per-row length: 6*21888 = 131328 = HWH+256
NPAD = NG * L - HWH         # 256 zero pads per channel per core
FB = 4096                   # main tile width
NMAIN = 5
TFB = L - NMAIN * FB        # 1408 = 11*128
NTILES = NMAIN + 1
ROWS = NG * C               # 120

FP32 = mybir.dt.float32
BF16 = mybir.dt.bfloat16
I8 = mybir.dt.int8
OP = mybir.AluOpType
AF = mybir.ActivationFunctionType

_CACHE = {}


def _build():
    """Build (and cache) the per-core bass program."""
    if "nc" in _CACHE:
        return _CACHE["nc"]

    nc = bacc.Bacc(
        "TRN2", target_bir_lowering=False, debug=False, num_devices=NCORES
    )

    x_d = nc.dram_tensor("x", [ROWS, L], FP32, kind="ExternalInput").ap()
    y_d = nc.dram_tensor("y", [ROWS, L], I8, kind="ExternalInput").ap()
    id_d = nc.dram_tensor("ident", [128, 128], BF16, kind="ExternalInput").ap()
    on_d = nc.dram_tensor("ones", [8, FB], BF16, kind="ExternalInput").ap()

    g_d = nc.dram_tensor("g_out", [128, 128], FP32, kind="ExternalOutput").ap()
    st_d = nc.dram_tensor("st_out", [128, 3 * NTILES], FP32, kind="ExternalOutput").ap()

    with tile.TileContext(nc) as tc, ExitStack() as ctx:
        sing = ctx.enter_context(tc.tile_pool(name="sing", bufs=1))
        xfpool = ctx.enter_context(tc.tile_pool(name="xfpool", bufs=2))
        xpool = ctx.enter_context(tc.tile_pool(name="xpool", bufs=2))
        ypool = ctx.enter_context(tc.tile_pool(name="ypool", bufs=2))
        epool = ctx.enter_context(tc.tile_pool(name="epool", bufs=2))
        spool = ctx.enter_context(tc.tile_pool(name="spool", bufs=3))
        pst_pool = ctx.enter_context(tc.tile_pool(name="pst", bufs=3, space="PSUM"))
        gp_pool = ctx.enter_context(tc.tile_pool(name="gp", bufs=1, space="PSUM"))

        ident = sing.tile([128, 128], BF16)
        nc.sync.dma_start(out=ident[:, :], in_=id_d)
        ones_sb = sing.tile([8, FB], BF16)
        nc.sync.dma_start(out=ones_sb[:, :], in_=on_d)

        # stats accumulator columns: [sabs | - | bce] per tile
        stats = sing.tile([128, 3 * NTILES], FP32)
        nc.vector.memset(stats[:, :], 0.0)
        mone = sing.tile([128, 1], FP32)
        nc.vector.memset(mone[:, :], -1.0)

        g_ps = gp_pool.tile([128, 121], FP32)

        # Load finite values into all 128x128 PE weight cells up front.
        warm = pst_pool.tile([128, 128], BF16)
        nc.tensor.transpose(out=warm[:, :], in_=ident[:, :], identity=ident[:, :])

        nmm = L // 128  # 171 total gram matmuls
        mm_count = 0

        for t in range(NTILES):
            fb = FB if t < NMAIN else TFB
            off = t * FB
            nsub = fb // 128

            # ---- X load: [120, fb] f32 HWDGE, then DVE cast -> bf16
            xf = xfpool.tile([ROWS, FB], FP32, tag="xf")
            nc.sync.dma_start(out=xf[:, 0:fb], in_=x_d[:, off : off + fb])
            xt = xpool.tile([128, FB], BF16, tag="xt")
            nc.vector.tensor_copy(out=xt[0:ROWS, 0:fb], in_=xf[:, 0:fb])
            nc.sync.dma_start(out=xt[ROWS:128, 0:fb], in_=ones_sb[:, 0:fb])

            # ---- Y load: [120, fb] int8 -> bf16 SWDGE cast DMA
            yt = ypool.tile([ROWS, FB], BF16, tag="yt")
            nc.gpsimd.dma_start(out=yt[:, 0:fb], in_=y_d[:, off : off + fb])

            # ---- s1 = x + y (bf16 TT, 2x mode)
            s1m1 = epool.tile([ROWS, FB], BF16, tag="s1m1")
            nc.vector.tensor_tensor(
                s1m1[:, 0:fb], xt[0:ROWS, 0:fb], yt[:, 0:fb], OP.add
            )

            # ---- |x+y-1| with free accumulate -> sum|x+y-1| (ACT, bias=-1)
            absz = epool.tile([ROWS, FB], BF16, tag="absz")
            nc.scalar.activation(
                out=absz[:, 0:fb],
                in_=s1m1[:, 0:fb],
                func=AF.Abs,
                bias=mone[0:ROWS, :],
                accum_out=stats[0:ROWS, 3 * t : 3 * t + 1],
            )

            # ---- bce partial: sum ln|x+y-1| (ACT with free accumulate)
            lnz = epool.tile([ROWS, FB], BF16, tag="lnz")
            nc.scalar.activation(
                out=lnz[:, 0:fb],
                in_=absz[:, 0:fb],
                func=AF.Ln,
                accum_out=stats[0:ROWS, 3 * t + 2 : 3 * t + 3],
            )

            # ---- gram: transpose each 128-col chunk, matmul-accumulate
            j = 0
            while j < nsub:
                gsz = min(4, nsub - j)
                pst = pst_pool.tile([128, 512], BF16, tag="pst")
                for jj in range(gsz):
                    nc.tensor.transpose(
                        out=pst[:, jj * 128 : (jj + 1) * 128],
                        in_=xt[:, (j + jj) * 128 : (j + jj + 1) * 128],
                        identity=ident[:, :],
                    )
                stsb = spool.tile([128, 512], BF16, tag="stsb")
                nc.vector.tensor_copy(
                    out=stsb[:, 0 : gsz * 128], in_=pst[:, 0 : gsz * 128]
                )
                for jj in range(gsz):
                    cof = jj * 128
                    mm_count += 1
                    nc.tensor.matmul(
                        out=g_ps[:, :],
                        lhsT=stsb[:, cof : cof + 128],
                        rhs=stsb[:, cof : cof + 121],
                        start=(mm_count == 1),
                        stop=(mm_count == nmm),
                        skip_group_check=True,
                    )
                j += gsz

        assert mm_count == nmm, mm_count

        # ---- write results out
        g_sb = sing.tile([128, 128], FP32)
        nc.vector.tensor_copy(out=g_sb[:, 0:121], in_=g_ps[:, :])
        nc.vector.memset(g_sb[:, 121:128], 0.0)
        nc.sync.dma_start(out=g_d, in_=g_sb[:, :])
        nc.sync.dma_start(out=st_d, in_=stats[:, :])

    nc.compile()
    _CACHE["nc"] = nc
    return nc


def _pack(core_slice, dtype):
    """[20, HWH] -> [120, L] slab: rows (g, c), zero-padded."""
    xp = np.zeros((C, NG * L), dtype=dtype)
    xp[:, :HWH] = core_slice
    return np.ascontiguousarray(
        xp.reshape(C, NG, L).transpose(1, 0, 2).reshape(ROWS, L)
    )


def _run(logit, label_lst, trace=False):
    nc = _build()
    X = np.asarray(logit, dtype=np.float32).reshape(B, C, HW)
    Y = np.asarray(label_lst).reshape(B, C, HW).astype(np.int8)
    ident = np.eye(128, dtype=ml_dtypes.bfloat16)
    ones = np.ones((8, FB), dtype=ml_dtypes.bfloat16)

    in_maps = []
    for k in range(NCORES):
        b, half = k // 2, k % 2
        sl = slice(half * HWH, (half + 1) * HWH)
        in_maps.append(
            {
                "x": _pack(X[b, :, sl], np.float32),
                "y": _pack(Y[b, :, sl], np.int8),
                "ident": ident,
                "ones": ones,
            }
        )
    return run_bass_kernel_spmd(nc, in_maps, list(range(NCORES)), trace=trace)


def _combine(results, sum_y):
    """Host-side tiny combine of per-core stats."""
    G = np.zeros((B, C, C), dtype=np.float64)
    sum_x = np.zeros((B, C), dtype=np.float64)
    sabs_r = np.zeros((B, C), dtype=np.float64)
    bce_r = np.zeros((B, C), dtype=np.float64)

    for k in range(NCORES):
        b = k // 2
        r = results[k]
        g = r["g_out"].astype(np.float64)
        st = r["st_out"].astype(np.float64)
        for gi in range(NG):
            sl = slice(gi * C, gi * C + C)
            G[b] += g[sl, sl]
            sum_x[b] += g[sl, 120]
        for t in range(NTILES):
            cols = st[:ROWS, 3 * t : 3 * t + 3].reshape(NG, C, 3)
            sabs_r[b] += cols[:, :, 0].sum(axis=0)
            bce_r[b] += cols[:, :, 2].sum(axis=0)

    # |x+y-1| = 2xy - x - y + 1 ; zero-pads count as x=0,y=0 elements.
    n_padded = HW + 2 * NPAD
    num = 0.5 * (sabs_r + sum_x + sum_y - n_padded)
    s = np.einsum("bii->bi", G)              # sum x^2

    # loss1
    numk = num[:, :KNOWN] + SMOOTH
    denk = s[:, :KNOWN] + sum_y[:, :KNOWN] + SMOOTH
    dice = np.mean(1.0 - numk / denk, axis=0)
    bce = -bce_r[:, :KNOWN].sum(axis=0) / (B * HW)
    loss1 = (dice + bce).sum() / KNOWN

    # loss2
    m = sum_x[:, KNOWN:].sum(axis=0) / (B * HW)
    loss2 = np.sum(-np.log(np.clip(m * 50.0, 1e-300, 1.0))) / (C - KNOWN)

    # loss3
    ratio = (G + SMOOTH) / (s[:, :, None] + s[:, None, :] + SMOOTH)
    M = ratio.mean(axis=0)
    loss3 = (M.sum() - np.trace(M)) / (C * (C - 1))

    loss = (loss1 + loss2 + loss3) * 0.1
    f = np.float32
    return f(loss), f(loss1), f(loss2), f(loss3)


def kernel(logit, label_lst, class_lst=None, **_):
    Yl = np.asarray(label_lst, dtype=np.int64).reshape(B, C, HW)
    sum_y = Yl.sum(axis=2).astype(np.float64)
    res = _run(logit, label_lst, trace=bool(os.environ.get("CDICE_TRACE")))
    out = _combine(res.results, sum_y)
    if os.environ.get("CDICE_TRACE"):
        kernel.last_result = res
    return out


# revision 16
# speedup vs baseline: 4.7193x; 1.2642x over previous
"""CDiceLoss Trainium2 kernel.

Shards B*HW over 8 cores (core = one (batch, half-of-HW) slice). The host
packs each core's logit slice into a [120, 21888] f32 "slab" (rows =
(group g, channel c), 6 groups per channel, zero-padded tail — x=0,y=0
pads are neutral in every statistic) and labels into the same layout as
int8. Per core the Tile kernel computes:
  - G     [128,121] PSUM gram: diag 20x20 blocks sum to sum_hw x_i x_j,
          ones-column 120 gives sum_x per row
  - sabs  = sum |x + y - 1|    ( = 2*sum(x*y) - sum x - sum y + n )
  - bce   = sum ln|x + y - 1|  ( = sum y*ln(x) + (1-y)*ln(1-x) )
sum_y is an exact host-side integer count; the host combines the tiny
per-core stats into (loss, loss1, loss2, loss3).
"""

import os
from contextlib import ExitStack

import numpy as np
import ml_dtypes

import concourse.bass as bass
import concourse.bacc as bacc
import concourse.tile as tile
from concourse import mybir
from concourse.bass_utils import run_bass_kernel_spmd

# ---------------- problem geometry (hardcoded) ----------------
B, C, H, W = 4, 20, 512, 512
HW = H * W                  # 262144
KNOWN = 16
SMOOTH = 1.0
NCORES = 8
HWH = HW // 2               # 131072 positions per core
NG = 6                      # channel-groups per slab
L = 21888                   # padded per-row length: 6*21888 = 131328 = HWH+256
NPAD = NG * L - HWH         # 256 zero pads per channel per core
FB = 4096                   # main tile width
NMAIN = 5
TFB = L - NMAIN * FB        # 1408 = 11*128
NTILES = NMAIN + 1
ROWS = NG * C               # 120

FP32 = mybir.dt.float32
BF16 = mybir.dt.bfloat16
I8 = mybir.dt.int8
OP = mybir.AluOpType
AF = mybir.ActivationFunctionType

_CACHE = {}


def _build():
    """Build (and cache) the per-core bass program."""
    if "nc" in _CACHE:
        return _CACHE["nc"]

    nc = bacc.Bacc(
        "TRN2", target_bir_lowering=False, debug=False, num_devices=NCORES
    )

    x_d = nc.dram_tensor("x", [ROWS, L], FP32, kind="ExternalInput").ap()
    y_d = nc.dram_tensor("y", [ROWS, L], I8, kind="ExternalInput").ap()
    id_d = nc.dram_tensor("ident", [128, 128], BF16, kind="ExternalInput").ap()
    on_d = nc.dram_tensor("ones", [8, FB], BF16, kind="ExternalInput").ap()

    g_d = nc.dram_tensor("g_out", [128, 128], FP32, kind="ExternalOutput").ap()
    st_d = nc.dram_tensor("st_out", [128, 3 * NTILES], FP32, kind="ExternalOutput").ap()

    with tile.TileContext(nc) as tc, ExitStack() as ctx:
        sing = ctx.enter_context(tc.tile_pool(name="sing", bufs=1))
        xpool = ctx.enter_context(tc.tile_pool(name="xpool", bufs=2))
        ypool = ctx.enter_context(tc.tile_pool(name="ypool", bufs=2))
        epool = ctx.enter_context(tc.tile_pool(name="epool", bufs=2))
        spool = ctx.enter_context(tc.tile_pool(name="spool", bufs=3))
        pst_pool = ctx.enter_context(tc.tile_pool(name="pst", bufs=3, space="PSUM"))
        gp_pool = ctx.enter_context(tc.tile_pool(name="gp", bufs=1, space="PSUM"))

        ident = sing.tile([128, 128], BF16)
        nc.sync.dma_start(out=ident[:, :], in_=id_d)
        ones_sb = sing.tile([8, FB], BF16)
        nc.sync.dma_start(out=ones_sb[:, :], in_=on_d)

        # stats accumulator columns: [sabs | - | bce] per tile
        stats = sing.tile([128, 3 * NTILES], FP32)
        nc.vector.memset(stats[:, :], 0.0)
        mone = sing.tile([128, 1], FP32)
        nc.vector.memset(mone[:, :], -1.0)

        g_ps = gp_pool.tile([128, 121], FP32)

        # Load finite values into all 128x128 PE weight cells up front.
        warm = pst_pool.tile([128, 128], BF16)
        nc.tensor.transpose(out=warm[:, :], in_=ident[:, :], identity=ident[:, :])

        nmm = L // 128  # 171 total gram matmuls
        mm_count = 0

        for t in range(NTILES):
            fb = FB if t < NMAIN else TFB
            off = t * FB
            nsub = fb // 128

            # ---- X load: [120, fb] f32 -> bf16 SWDGE cast DMA (2-D AP)
            xt = xpool.tile([128, FB], BF16, tag="xt")
            nc.gpsimd.dma_start(out=xt[0:ROWS, 0:fb], in_=x_d[:, off : off + fb])
            nc.sync.dma_start(out=xt[ROWS:128, 0:fb], in_=ones_sb[:, 0:fb])

            # ---- Y load: [120, fb] int8 -> bf16 SWDGE cast DMA
            yt = ypool.tile([ROWS, FB], BF16, tag="yt")
            nc.gpsimd.dma_start(out=yt[:, 0:fb], in_=y_d[:, off : off + fb])

            # ---- s1 = x + y (bf16 TT, 2x mode)
            s1m1 = epool.tile([ROWS, FB], BF16, tag="s1m1")
            nc.vector.tensor_tensor(
                s1m1[:, 0:fb], xt[0:ROWS, 0:fb], yt[:, 0:fb], OP.add
            )

            # ---- |x+y-1| with free accumulate -> sum|x+y-1| (ACT, bias=-1)
            absz = epool.tile([ROWS, FB], BF16, tag="absz")
            nc.scalar.activation(
                out=absz[:, 0:fb],
                in_=s1m1[:, 0:fb],
                func=AF.Abs,
                bias=mone[0:ROWS, :],
                accum_out=stats[0:ROWS, 3 * t : 3 * t + 1],
            )

            # ---- bce partial: sum ln|x+y-1| (ACT with free accumulate)
            lnz = epool.tile([ROWS, FB], BF16, tag="lnz")
            nc.scalar.activation(
                out=lnz[:, 0:fb],
                in_=absz[:, 0:fb],
                func=AF.Ln,
                accum_out=stats[0:ROWS, 3 * t + 2 : 3 * t + 3],
            )

            # ---- gram: transpose each 128-col chunk, matmul-accumulate
            j = 0
            while j < nsub:
                gsz = min(4, nsub - j)
                pst = pst_pool.tile([128, 512], BF16, tag="pst")
                for jj in range(gsz):
                    nc.tensor.transpose(
                        out=pst[:, jj * 128 : (jj + 1) * 128],
                        in_=xt[:, (j + jj) * 128 : (j + jj + 1) * 128],
                        identity=ident[:, :],
                    )
                stsb = spool.tile([128, 512], BF16, tag="stsb")
                nc.vector.tensor_copy(
                    out=stsb[:, 0 : gsz * 128], in_=pst[:, 0 : gsz * 128]
                )
                for jj in range(gsz):
                    cof = jj * 128
                    mm_count += 1
                    nc.tensor.matmul(
                        out=g_ps[:, :],
                        lhsT=stsb[:, cof : cof + 128],
                        rhs=stsb[:, cof : cof + 121],
                        start=(mm_count == 1),
                        stop=(mm_count == nmm),
                        skip_group_check=True,
                    )
                j += gsz

        assert mm_count == nmm, mm_count

        # ---- write results out
        g_sb = sing.tile([128, 128], FP32)
        nc.vector.tensor_copy(out=g_sb[:, 0:121], in_=g_ps[:, :])
        nc.vector.memset(g_sb[:, 121:128], 0.0)
        nc.sync.dma_start(out=g_d, in_=g_sb[:, :])
        nc.sync.dma_start(out=st_d, in_=stats[:, :])

    nc.compile()
    _CACHE["nc"] = nc
    return nc


def _pack(core_slice, dtype):
    """[20, HWH] -> [120, L] slab: rows (g, c), zero-padded."""
    xp = np.zeros((C, NG * L), dtype=dtype)
    xp[:, :HWH] = core_slice
    return np.ascontiguousarray(
        xp.reshape(C, NG, L).transpose(1, 0, 2).reshape(ROWS, L)
    )


def _run(logit, label_lst, trace=False):
    nc = _build()
    X = np.asarray(logit, dtype=np.float32).reshape(B, C, HW)
    Y = np.asarray(label_lst).reshape(B, C, HW).astype(np.int8)
    ident = np.eye(128, dtype=ml_dtypes.bfloat16)
    ones = np.ones((8, FB), dtype=ml_dtypes.bfloat16)

    in_maps = []
    for k in range(NCORES):
        b, half = k // 2, k % 2
        sl = slice(half * HWH, (half + 1) * HWH)
        in_maps.append(
            {
                "x": _pack(X[b, :, sl], np.float32),
                "y": _pack(Y[b, :, sl], np.int8),
                "ident": ident,
                "ones": ones,
            }
        )
    return run_bass_kernel_spmd(nc, in_maps, list(range(NCORES)), trace=trace)


def _combine(results, sum_y):
    """Host-side tiny combine of per-core stats."""
    G = np.zeros((B, C, C), dtype=np.float64)
    sum_x = np.zeros((B, C), dtype=np.float64)
    sabs_r = np.zeros((B, C), dtype=np.float64)
    bce_r = np.zeros((B, C), dtype=np.float64)

    for k in range(NCORES):
        b = k // 2
        r = results[k]
        g = r["g_out"].astype(np.float64)
        st = r["st_out"].astype(np.float64)
        for gi in range(NG):
            sl = slice(gi * C, gi * C + C)
            G[b] += g[sl, sl]
            sum_x[b] += g[sl, 120]
        for t in range(NTILES):
            cols = st[:ROWS, 3 * t : 3 * t + 3].reshape(NG, C, 3)
            sabs_r[b] += cols[:, :, 0].sum(axis=0)
            bce_r[b] += cols[:, :, 2].sum(axis=0)

    # |x+y-1| = 2xy - x - y + 1 ; zero-pads count as x=0,y=0 elements.
    n_padded = HW + 2 * NPAD
    num = 0.5 * (sabs_r + sum_x + sum_y - n_padded)
    s = np.einsum("bii->bi", G)              # sum x^2

    # loss1
    numk = num[:, :KNOWN] + SMOOTH
    denk = s[:, :KNOWN] + sum_y[:, :KNOWN] + SMOOTH
    dice = np.mean(1.0 - numk / denk, axis=0)
    bce = -bce_r[:, :KNOWN].sum(axis=0) / (B * HW)
    loss1 = (dice + bce).sum() / KNOWN

    # loss2
    m = sum_x[:, KNOWN:].sum(axis=0) / (B * HW)
    loss2 = np.sum(-np.log(np.clip(m * 50.0, 1e-300, 1.0))) / (C - KNOWN)

    # loss3
    ratio = (G + SMOOTH) / (s[:, :, None] + s[:, None, :] + SMOOTH)
    M = ratio.mean(axis=0)
    loss3 = (M.sum() - np.trace(M)) / (C * (C - 1))

    loss = (loss1 + loss2 + loss3) * 0.1
    f = np.float32
    return f(loss), f(loss1), f(loss2), f(loss3)


def kernel(logit, label_lst, class_lst=None, **_):
    Yl = np.asarray(label_lst, dtype=np.int64).reshape(B, C, HW)
    sum_y = Yl.sum(axis=2).astype(np.float64)
    res = _run(logit, label_lst, trace=bool(os.environ.get("CDICE_TRACE")))
    out = _combine(res.results, sum_y)
    if os.environ.get("CDICE_TRACE"):
        kernel.last_result = res
    return out


# revision 17
# speedup vs baseline: 4.9347x; 1.0456x over previous
"""CDiceLoss Trainium2 kernel.

Shards B*HW over 8 cores (core = one (batch, half-of-HW) slice). The host
packs each core's logit slice into a [120, 21888] f32 "slab" (rows =
(group g, channel c), 6 groups per channel, zero-padded tail — x=0,y=0
pads are neutral in every statistic) and labels into the same layout as
int8. Per core the Tile kernel computes:
  - G     [128,121] PSUM gram: diag 20x20 blocks sum to sum_hw x_i x_j,
          ones-column 120 gives sum_x per row
  - sabs  = sum |x + y - 1|    ( = 2*sum(x*y) - sum x - sum y + n )
  - bce   = sum ln|x + y - 1|  ( = sum y*ln(x) + (1-y)*ln(1-x) )
sum_y is an exact host-side integer count; the host combines the tiny
per-core stats into (loss, loss1, loss2, loss3).
"""

import os
from contextlib import ExitStack

import numpy as np
import ml_dtypes

import concourse.bass as bass
import concourse.bacc as bacc
import concourse.tile as tile
from concourse import mybir
from concourse.bass_utils import run_bass_kernel_spmd

# ---------------- problem geometry (hardcoded) ----------------
B, C, H, W = 4, 20, 512, 512
HW = H * W                  # 262144
KNOWN = 16
SMOOTH = 1.0
NCORES = 8
HWH = HW // 2               # 131072 positions per core
NG = 6                      # channel-groups per slab
L = 21888                   # padded per-row length: 6*21888 = 131328 = HWH+256
NPAD = NG * L - HWH         # 256 zero pads per channel per core
FB = 4096                   # main tile width
NMAIN = 5
TFB = L - NMAIN * FB        # 1408 = 11*128
NTILES = NMAIN + 1
ROWS = NG * C               # 120

FP32 = mybir.dt.float32
BF16 = mybir.dt.bfloat16
I8 = mybir.dt.int8
OP = mybir.AluOpType
AF = mybir.ActivationFunctionType

_CACHE = {}


def _build():
    """Build (and cache) the per-core bass program."""
    if "nc" in _CACHE:
        return _CACHE["nc"]

    nc = bacc.Bacc(
        "TRN2", target_bir_lowering=False, debug=False, num_devices=NCORES
    )

    x_d = nc.dram_tensor("x", [128, L], FP32, kind="ExternalInput").ap()
    y_d = nc.dram_tensor("y", [ROWS, L], I8, kind="ExternalInput").ap()
    id_d = nc.dram_tensor("ident", [128, 128], BF16, kind="ExternalInput").ap()

    g_d = nc.dram_tensor("g_out", [128, 128], FP32, kind="ExternalOutput").ap()
    st_d = nc.dram_tensor("st_out", [128, 3 * NTILES], FP32, kind="ExternalOutput").ap()

    with tile.TileContext(nc) as tc, ExitStack() as ctx:
        sing = ctx.enter_context(tc.tile_pool(name="sing", bufs=1))
        xpool = ctx.enter_context(tc.tile_pool(name="xpool", bufs=3))
        ypool = ctx.enter_context(tc.tile_pool(name="ypool", bufs=3))
        epool = ctx.enter_context(tc.tile_pool(name="epool", bufs=2))
        spool = ctx.enter_context(tc.tile_pool(name="spool", bufs=3))
        pst_pool = ctx.enter_context(tc.tile_pool(name="pst", bufs=3, space="PSUM"))
        gp_pool = ctx.enter_context(tc.tile_pool(name="gp", bufs=1, space="PSUM"))

        ident = sing.tile([128, 128], BF16)
        nc.sync.dma_start(out=ident[:, :], in_=id_d)

        # stats accumulator columns: [sabs | - | bce] per tile
        stats = sing.tile([128, 3 * NTILES], FP32)
        nc.vector.memset(stats[:, :], 0.0)
        mone = sing.tile([128, 1], FP32)
        nc.vector.memset(mone[:, :], -1.0)

        g_ps = gp_pool.tile([128, 121], FP32)

        # Load finite values into all 128x128 PE weight cells up front.
        warm = pst_pool.tile([128, 128], BF16)
        nc.tensor.transpose(out=warm[:, :], in_=ident[:, :], identity=ident[:, :])

        nmm = L // 128  # 171 total gram matmuls
        mm_count = 0

        for t in range(NTILES):
            fb = FB if t < NMAIN else TFB
            off = t * FB
            nsub = fb // 128

            # ---- X load: [128, fb] f32 -> bf16 SWDGE cast DMA (2-D AP);
            # rows 120-127 are host-baked ones (the gram's sum column).
            xt = xpool.tile([128, FB], BF16, tag="xt")
            nc.gpsimd.dma_start(out=xt[:, 0:fb], in_=x_d[:, off : off + fb])

            # ---- Y load: [120, fb] int8 -> bf16 SWDGE cast DMA
            yt = ypool.tile([ROWS, FB], BF16, tag="yt")
            nc.gpsimd.dma_start(out=yt[:, 0:fb], in_=y_d[:, off : off + fb])

            # ---- s1 = x + y (bf16 TT, 2x mode)
            s1m1 = epool.tile([ROWS, FB], BF16, tag="s1m1")
            nc.vector.tensor_tensor(
                s1m1[:, 0:fb], xt[0:ROWS, 0:fb], yt[:, 0:fb], OP.add
            )

            # ---- |x+y-1| with free accumulate -> sum|x+y-1| (ACT, bias=-1)
            absz = epool.tile([ROWS, FB], BF16, tag="absz")
            nc.scalar.activation(
                out=absz[:, 0:fb],
                in_=s1m1[:, 0:fb],
                func=AF.Abs,
                bias=mone[0:ROWS, :],
                accum_out=stats[0:ROWS, 3 * t : 3 * t + 1],
            )

            # ---- bce partial: sum ln|x+y-1| (ACT with free accumulate)
            lnz = epool.tile([ROWS, FB], BF16, tag="lnz")
            nc.scalar.activation(
                out=lnz[:, 0:fb],
                in_=absz[:, 0:fb],
                func=AF.Ln,
                accum_out=stats[0:ROWS, 3 * t + 2 : 3 * t + 3],
            )

            # ---- gram: transpose each 128-col chunk, matmul-accumulate
            j = 0
            while j < nsub:
                gsz = min(4, nsub - j)
                pst = pst_pool.tile([128, 512], BF16, tag="pst")
                for jj in range(gsz):
                    nc.tensor.transpose(
                        out=pst[:, jj * 128 : (jj + 1) * 128],
                        in_=xt[:, (j + jj) * 128 : (j + jj + 1) * 128],
                        identity=ident[:, :],
                    )
                stsb = spool.tile([128, 512], BF16, tag="stsb")
                nc.vector.tensor_copy(
                    out=stsb[:, 0 : gsz * 128], in_=pst[:, 0 : gsz * 128]
                )
                for jj in range(gsz):
                    cof = jj * 128
                    mm_count += 1
                    nc.tensor.matmul(
                        out=g_ps[:, :],
                        lhsT=stsb[:, cof : cof + 128],
                        rhs=stsb[:, cof : cof + 121],
                        start=(mm_count == 1),
                        stop=(mm_count == nmm),
                        skip_group_check=True,
                    )
                j += gsz

        assert mm_count == nmm, mm_count

        # ---- write results out
        g_sb = sing.tile([128, 128], FP32)
        nc.vector.tensor_copy(out=g_sb[:, 0:121], in_=g_ps[:, :])
        nc.vector.memset(g_sb[:, 121:128], 0.0)
        nc.sync.dma_start(out=g_d, in_=g_sb[:, :])
        nc.sync.dma_start(out=st_d, in_=stats[:, :])

    nc.compile()
    _CACHE["nc"] = nc
    return nc


def _pack(core_slice, dtype, ones_rows=False):
    """[20, HWH] -> [120(+8), L] slab: rows (g, c), zero-padded."""
    nr = 128 if ones_rows else ROWS
    out = np.empty((nr, L), dtype=dtype)
    xp = np.zeros((C, NG * L), dtype=dtype)
    xp[:, :HWH] = core_slice
    out[:ROWS] = xp.reshape(C, NG, L).transpose(1, 0, 2).reshape(ROWS, L)
    if ones_rows:
        out[ROWS:] = 1
    return np.ascontiguousarray(out)


def _run(logit, label_lst, trace=False):
    nc = _build()
    X = np.asarray(logit, dtype=np.float32).reshape(B, C, HW)
    Y = np.asarray(label_lst).reshape(B, C, HW).astype(np.int8)
    ident = np.eye(128, dtype=ml_dtypes.bfloat16)

    in_maps = []
    for k in range(NCORES):
        b, half = k // 2, k % 2
        sl = slice(half * HWH, (half + 1) * HWH)
        in_maps.append(
            {
                "x": _pack(X[b, :, sl], np.float32, ones_rows=True),
                "y": _pack(Y[b, :, sl], np.int8),
                "ident": ident,
            }
        )
    return run_bass_kernel_spmd(nc, in_maps, list(range(NCORES)), trace=trace)


def _combine(results, sum_y):
    """Host-side tiny combine of per-core stats."""
    G = np.zeros((B, C, C), dtype=np.float64)
    sum_x = np.zeros((B, C), dtype=np.float64)
    sabs_r = np.zeros((B, C), dtype=np.float64)
    bce_r = np.zeros((B, C), dtype=np.float64)

    for k in range(NCORES):
        b = k // 2
        r = results[k]
        g = r["g_out"].astype(np.float64)
        st = r["st_out"].astype(np.float64)
        for gi in range(NG):
            sl = slice(gi * C, gi * C + C)
            G[b] += g[sl, sl]
            sum_x[b] += g[sl, 120]
        for t in range(NTILES):
            cols = st[:ROWS, 3 * t : 3 * t + 3].reshape(NG, C, 3)
            sabs_r[b] += cols[:, :, 0].sum(axis=0)
            bce_r[b] += cols[:, :, 2].sum(axis=0)

    # |x+y-1| = 2xy - x - y + 1 ; zero-pads count as x=0,y=0 elements.
    n_padded = HW + 2 * NPAD
    num = 0.5 * (sabs_r + sum_x + sum_y - n_padded)
    s = np.einsum("bii->bi", G)              # sum x^2

    # loss1
    numk = num[:, :KNOWN] + SMOOTH
    denk = s[:, :KNOWN] + sum_y[:, :KNOWN] + SMOOTH
    dice = np.mean(1.0 - numk / denk, axis=0)
    bce = -bce_r[:, :KNOWN].sum(axis=0) / (B * HW)
    loss1 = (dice + bce).sum() / KNOWN

    # loss2
    m = sum_x[:, KNOWN:].sum(axis=0) / (B * HW)
    loss2 = np.sum(-np.log(np.clip(m * 50.0, 1e-300, 1.0))) / (C - KNOWN)

    # loss3
    ratio = (G + SMOOTH) / (s[:, :, None] + s[:, None, :] + SMOOTH)
    M = ratio.mean(axis=0)
    loss3 = (M.sum() - np.trace(M)) / (C * (C - 1))

    loss = (loss1 + loss2 + loss3) * 0.1
    f = np.float32
    return f(loss), f(loss1), f(loss2), f(loss3)


def kernel(logit, label_lst, class_lst=None, **_):
    Yl = np.asarray(label_lst, dtype=np.int64).reshape(B, C, HW)
    sum_y = Yl.sum(axis=2).astype(np.float64)
    res = _run(logit, label_lst, trace=bool(os.environ.get("CDICE_TRACE")))
    out = _combine(res.results, sum_y)
    if os.environ.get("CDICE_TRACE"):
        kernel.last_result = res
    return out


# revision 18
# speedup vs baseline: 5.4506x; 1.1045x over previous
"""CDiceLoss Trainium2 kernel.

Shards B*HW over 8 cores (core = one (batch, half-of-HW) slice). The host
packs each core's logit slice into a [120, 21888] f32 "slab" (rows =
(group g, channel c), 6 groups per channel, zero-padded tail — x=0,y=0
pads are neutral in every statistic) and labels into the same layout as
int8. Per core the Tile kernel computes:
  - G     [128,121] PSUM gram: diag 20x20 blocks sum to sum_hw x_i x_j,
          ones-column 120 gives sum_x per row
  - sabs  = sum |x + y - 1|    ( = 2*sum(x*y) - sum x - sum y + n )
  - bce   = sum ln|x + y - 1|  ( = sum y*ln(x) + (1-y)*ln(1-x) )
sum_y is an exact host-side integer count; the host combines the tiny
per-core stats into (loss, loss1, loss2, loss3).
"""

import os
from contextlib import ExitStack

import numpy as np
import ml_dtypes

import concourse.bass as bass
import concourse.bacc as bacc
import concourse.tile as tile
from concourse import mybir
from concourse.bass_utils import run_bass_kernel_spmd

# ---------------- problem geometry (hardcoded) ----------------
B, C, H, W = 4, 20, 512, 512
HW = H * W                  # 262144
KNOWN = 16
SMOOTH = 1.0
NCORES = 8
HWH = HW // 2               # 131072 positions per core
NG = 6                      # channel-groups per slab
L = 21888                   # padded per-row length: 6*21888 = 131328 = HWH+256
NPAD = NG * L - HWH         # 256 zero pads per channel per core
FB = 2048                   # main tile width
NMAIN = 10
TFB = L - NMAIN * FB        # 1408 = 11*128
NTILES = NMAIN + 1
ROWS = NG * C               # 120

FP32 = mybir.dt.float32
BF16 = mybir.dt.bfloat16
I8 = mybir.dt.int8
OP = mybir.AluOpType
AF = mybir.ActivationFunctionType

_CACHE = {}


def _build():
    """Build (and cache) the per-core bass program."""
    if "nc" in _CACHE:
        return _CACHE["nc"]

    nc = bacc.Bacc(
        "TRN2", target_bir_lowering=False, debug=False, num_devices=NCORES
    )

    x_d = nc.dram_tensor("x", [128, L], FP32, kind="ExternalInput").ap()
    y_d = nc.dram_tensor("y", [ROWS, L], I8, kind="ExternalInput").ap()
    id_d = nc.dram_tensor("ident", [128, 128], BF16, kind="ExternalInput").ap()

    g_d = nc.dram_tensor("g_out", [128, 128], FP32, kind="ExternalOutput").ap()
    st_d = nc.dram_tensor("st_out", [128, 3 * NTILES], FP32, kind="ExternalOutput").ap()

    with tile.TileContext(nc) as tc, ExitStack() as ctx:
        sing = ctx.enter_context(tc.tile_pool(name="sing", bufs=1))
        xpool = ctx.enter_context(tc.tile_pool(name="xpool", bufs=3))
        ypool = ctx.enter_context(tc.tile_pool(name="ypool", bufs=3))
        epool = ctx.enter_context(tc.tile_pool(name="epool", bufs=3))
        spool = ctx.enter_context(tc.tile_pool(name="spool", bufs=4))
        pst_pool = ctx.enter_context(tc.tile_pool(name="pst", bufs=4, space="PSUM"))
        gp_pool = ctx.enter_context(tc.tile_pool(name="gp", bufs=1, space="PSUM"))

        ident = sing.tile([128, 128], BF16)
        nc.sync.dma_start(out=ident[:, :], in_=id_d)

        # stats accumulator columns: [sabs | - | bce] per tile
        stats = sing.tile([128, 3 * NTILES], FP32)
        nc.vector.memset(stats[:, :], 0.0)
        mone = sing.tile([128, 1], FP32)
        nc.vector.memset(mone[:, :], -1.0)

        g_ps = gp_pool.tile([128, 121], FP32)

        # Load finite values into all 128x128 PE weight cells, then run a
        # burst of dummy matmuls during the first DMA to trip the PE HAM
        # clock-gate to 8/8 before real grams arrive.
        warm = pst_pool.tile([128, 128], BF16, tag="pst")
        nc.tensor.transpose(out=warm[:, :], in_=ident[:, :], identity=ident[:, :])
        wps = gp_pool.tile([128, 128], FP32)
        for _ in range(48):
            nc.tensor.matmul(
                out=wps[:, :], lhsT=ident[:, :], rhs=ident[:, :],
                start=True, stop=True, skip_group_check=True,
            )

        nmm = L // 128  # 171 total gram matmuls
        mm_count = 0

        for t in range(NTILES):
            fb = FB if t < NMAIN else TFB
            off = t * FB
            nsub = fb // 128

            # ---- X load: [128, fb] f32 -> bf16 SWDGE cast DMA (2-D AP);
            # rows 120-127 are host-baked ones (the gram's sum column).
            xt = xpool.tile([128, FB], BF16, tag="xt")
            nc.gpsimd.dma_start(out=xt[:, 0:fb], in_=x_d[:, off : off + fb])

            # ---- Y load: [120, fb] int8 -> bf16 SWDGE cast DMA
            yt = ypool.tile([ROWS, FB], BF16, tag="yt")
            nc.gpsimd.dma_start(out=yt[:, 0:fb], in_=y_d[:, off : off + fb])

            # ---- s1 = x + y (bf16 TT, 2x mode)
            s1m1 = epool.tile([ROWS, FB], BF16, tag="s1m1")
            nc.vector.tensor_tensor(
                s1m1[:, 0:fb], xt[0:ROWS, 0:fb], yt[:, 0:fb], OP.add
            )

            # ---- |x+y-1| with free accumulate -> sum|x+y-1| (ACT, bias=-1)
            absz = epool.tile([ROWS, FB], BF16, tag="absz")
            nc.scalar.activation(
                out=absz[:, 0:fb],
                in_=s1m1[:, 0:fb],
                func=AF.Abs,
                bias=mone[0:ROWS, :],
                accum_out=stats[0:ROWS, 3 * t : 3 * t + 1],
            )

            # ---- bce partial: sum ln|x+y-1| (ACT with free accumulate)
            lnz = epool.tile([ROWS, FB], BF16, tag="lnz")
            nc.scalar.activation(
                out=lnz[:, 0:fb],
                in_=absz[:, 0:fb],
                func=AF.Ln,
                accum_out=stats[0:ROWS, 3 * t + 2 : 3 * t + 3],
            )

            # ---- gram: transpose each 128-col chunk, matmul-accumulate
            j = 0
            while j < nsub:
                gsz = min(4, nsub - j)
                pst = pst_pool.tile([128, 512], BF16, tag="pst")
                for jj in range(gsz):
                    nc.tensor.transpose(
                        out=pst[:, jj * 128 : (jj + 1) * 128],
                        in_=xt[:, (j + jj) * 128 : (j + jj + 1) * 128],
                        identity=ident[:, :],
                    )
                stsb = spool.tile([128, 512], BF16, tag="stsb")
                nc.vector.tensor_copy(
                    out=stsb[:, 0 : gsz * 128], in_=pst[:, 0 : gsz * 128]
                )
                for jj in range(gsz):
                    cof = jj * 128
                    mm_count += 1
                    nc.tensor.matmul(
                        out=g_ps[:, :],
                        lhsT=stsb[:, cof : cof + 128],
                        rhs=stsb[:, cof : cof + 121],
                        start=(mm_count == 1),
                        stop=(mm_count == nmm),
                        skip_group_check=True,
                    )
                j += gsz

        assert mm_count == nmm, mm_count

        # ---- write results out
        g_sb = sing.tile([128, 128], FP32)
        nc.vector.tensor_copy(out=g_sb[:, 0:121], in_=g_ps[:, :])
        nc.vector.memset(g_sb[:, 121:128], 0.0)
        nc.sync.dma_start(out=g_d, in_=g_sb[:, :])
        nc.sync.dma_start(out=st_d, in_=stats[:, :])

    nc.compile()
    _CACHE["nc"] = nc
    return nc


def _pack(core_slice, dtype, ones_rows=False):
    """[20, HWH] -> [120(+8), L] slab: rows (g, c), zero-padded."""
    nr = 128 if ones_rows else ROWS
    out = np.empty((nr, L), dtype=dtype)
    xp = np.zeros((C, NG * L), dtype=dtype)
    xp[:, :HWH] = core_slice
    out[:ROWS] = xp.reshape(C, NG, L).transpose(1, 0, 2).reshape(ROWS, L)
    if ones_rows:
        out[ROWS:] = 1
    return np.ascontiguousarray(out)


def _run(logit, label_lst, trace=False):
    nc = _build()
    X = np.asarray(logit, dtype=np.float32).reshape(B, C, HW)
    Y = np.asarray(label_lst).reshape(B, C, HW).astype(np.int8)
    ident = np.eye(128, dtype=ml_dtypes.bfloat16)

    in_maps = []
    for k in range(NCORES):
        b, half = k // 2, k % 2
        sl = slice(half * HWH, (half + 1) * HWH)
        in_maps.append(
            {
                "x": _pack(X[b, :, sl], np.float32, ones_rows=True),
                "y": _pack(Y[b, :, sl], np.int8),
                "ident": ident,
            }
        )
    return run_bass_kernel_spmd(nc, in_maps, list(range(NCORES)), trace=trace)


def _combine(results, sum_y):
    """Host-side tiny combine of per-core stats."""
    G = np.zeros((B, C, C), dtype=np.float64)
    sum_x = np.zeros((B, C), dtype=np.float64)
    sabs_r = np.zeros((B, C), dtype=np.float64)
    bce_r = np.zeros((B, C), dtype=np.float64)

    for k in range(NCORES):
        b = k // 2
        r = results[k]
        g = r["g_out"].astype(np.float64)
        st = r["st_out"].astype(np.float64)
        for gi in range(NG):
            sl = slice(gi * C, gi * C + C)
            G[b] += g[sl, sl]
            sum_x[b] += g[sl, 120]
        for t in range(NTILES):
            cols = st[:ROWS, 3 * t : 3 * t + 3].reshape(NG, C, 3)
            sabs_r[b] += cols[:, :, 0].sum(axis=0)
            bce_r[b] += cols[:, :, 2].sum(axis=0)

    # |x+y-1| = 2xy - x - y + 1 ; zero-pads count as x=0,y=0 elements.
    n_padded = HW + 2 * NPAD
    num = 0.5 * (sabs_r + sum_x + sum_y - n_padded)
    s = np.einsum("bii->bi", G)              # sum x^2

    # loss1
    numk = num[:, :KNOWN] + SMOOTH
    denk = s[:, :KNOWN] + sum_y[:, :KNOWN] + SMOOTH
    dice = np.mean(1.0 - numk / denk, axis=0)
    bce = -bce_r[:, :KNOWN].sum(axis=0) / (B * HW)
    loss1 = (dice + bce).sum() / KNOWN

    # loss2
    m = sum_x[:, KNOWN:].sum(axis=0) / (B * HW)
    loss2 = np.sum(-np.log(np.clip(m * 50.0, 1e-300, 1.0))) / (C - KNOWN)

    # loss3
    ratio = (G + SMOOTH) / (s[:, :, None] + s[:, None, :] + SMOOTH)
    M = ratio.mean(axis=0)
    loss3 = (M.sum() - np.trace(M)) / (C * (C - 1))

    loss = (loss1 + loss2 + loss3) * 0.1
    f = np.float32
    return f(loss), f(loss1), f(loss2), f(loss3)


def kernel(logit, label_lst, class_lst=None, **_):
    Yl = np.asarray(label_lst, dtype=np.int64).reshape(B, C, HW)
    sum_y = Yl.sum(axis=2).astype(np.float64)
    res = _run(logit, label_lst, trace=bool(os.environ.get("CDICE_TRACE")))
    out = _combine(res.results, sum_y)
    if os.environ.get("CDICE_TRACE"):
        kernel.last_result = res
    return out


# revision 19
# speedup vs baseline: 5.9484x; 1.0913x over previous
"""CDiceLoss Trainium2 kernel.

Shards B*HW over 8 cores (core = one (batch, half-of-HW) slice). The host
packs each core's logit slice into a [120, 21888] f32 "slab" (rows =
(group g, channel c), 6 groups per channel, zero-padded tail — x=0,y=0
pads are neutral in every statistic) and labels into the same layout as
int8. Per core the Tile kernel computes:
  - G     [128,121] PSUM gram: diag 20x20 blocks sum to sum_hw x_i x_j,
          ones-column 120 gives sum_x per row
  - sabs  = sum |x + y - 1|    ( = 2*sum(x*y) - sum x - sum y + n )
  - bce   = sum ln|x + y - 1|  ( = sum y*ln(x) + (1-y)*ln(1-x) )
sum_y is an exact host-side integer count; the host combines the tiny
per-core stats into (loss, loss1, loss2, loss3).
"""

import os
from contextlib import ExitStack

import numpy as np
import ml_dtypes

import concourse.bass as bass
import concourse.bacc as bacc
import concourse.tile as tile
from concourse import mybir
from concourse.bass_utils import run_bass_kernel_spmd

# ---------------- problem geometry (hardcoded) ----------------
B, C, H, W = 4, 20, 512, 512
HW = H * W                  # 262144
KNOWN = 16
SMOOTH = 1.0
NCORES = 8
HWH = HW // 2               # 131072 positions per core
NG = 6                      # channel-groups per slab
L = 21888                   # padded per-row length: 6*21888 = 131328 = HWH+256
NPAD = NG * L - HWH         # 256 zero pads per channel per core
FB = 2048                   # main tile width
NMAIN = 10
TFB = L - NMAIN * FB        # 1408 = 11*128
NTILES = NMAIN + 1
ROWS = NG * C               # 120

FP32 = mybir.dt.float32
BF16 = mybir.dt.bfloat16
I8 = mybir.dt.int8
OP = mybir.AluOpType
AF = mybir.ActivationFunctionType

_CACHE = {}


def _build():
    """Build (and cache) the per-core bass program."""
    if "nc" in _CACHE:
        return _CACHE["nc"]

    nc = bacc.Bacc(
        "TRN2", target_bir_lowering=False, debug=False, num_devices=NCORES
    )

    x_d = nc.dram_tensor("x", [128, L], FP32, kind="ExternalInput").ap()
    y_d = nc.dram_tensor("y", [ROWS, L], I8, kind="ExternalInput").ap()
    id_d = nc.dram_tensor("ident", [128, 128], BF16, kind="ExternalInput").ap()

    g_d = nc.dram_tensor("g_out", [128, 128], FP32, kind="ExternalOutput").ap()
    st_d = nc.dram_tensor("st_out", [128, 3 * NTILES], FP32, kind="ExternalOutput").ap()

    with tile.TileContext(nc) as tc, ExitStack() as ctx:
        sing = ctx.enter_context(tc.tile_pool(name="sing", bufs=1))
        xpool = ctx.enter_context(tc.tile_pool(name="xpool", bufs=3))
        ypool = ctx.enter_context(tc.tile_pool(name="ypool", bufs=3))
        epool = ctx.enter_context(tc.tile_pool(name="epool", bufs=3))
        spool = ctx.enter_context(tc.tile_pool(name="spool", bufs=4))
        pst_pool = ctx.enter_context(tc.tile_pool(name="pst", bufs=4, space="PSUM"))
        gp_pool = ctx.enter_context(tc.tile_pool(name="gp", bufs=1, space="PSUM"))

        ident = sing.tile([128, 128], BF16)
        nc.sync.dma_start(out=ident[:, :], in_=id_d)

        # stats accumulator columns: [sabs | - | bce] per tile
        stats = sing.tile([128, 3 * NTILES], FP32)
        nc.vector.memset(stats[:, :], 0.0)
        mone = sing.tile([128, 1], FP32)
        nc.vector.memset(mone[:, :], -1.0)

        g_ps = gp_pool.tile([128, 121], FP32)

        # Load finite values into all 128x128 PE weight cells, then run a
        # burst of dummy matmuls during the first DMA to trip the PE HAM
        # clock-gate to 8/8 before real grams arrive.
        warm = pst_pool.tile([128, 128], BF16, tag="pst")
        nc.tensor.transpose(out=warm[:, :], in_=ident[:, :], identity=ident[:, :])
        wps = gp_pool.tile([128, 128], FP32)
        for _ in range(48):
            nc.tensor.matmul(
                out=wps[:, :], lhsT=ident[:, :], rhs=ident[:, :],
                start=True, stop=True, skip_group_check=True,
            )

        nmm = L // 128  # 171 total gram matmuls
        mm_count = 0

        for t in range(NTILES):
            fb = FB if t < NMAIN else TFB
            off = t * FB
            nsub = fb // 128

            # ---- X load: [128, fb] f32 -> bf16 SWDGE cast DMA (2-D AP);
            # rows 120-127 are host-baked ones (the gram's sum column).
            xt = xpool.tile([128, FB], BF16, tag="xt")
            nc.gpsimd.dma_start(out=xt[:, 0:fb], in_=x_d[:, off : off + fb])

            # ---- Y load: [120, fb] raw int8 (HWDGE; half the write bytes)
            yt = ypool.tile([ROWS, FB], I8, tag="yt")
            nc.sync.dma_start(out=yt[:, 0:fb], in_=y_d[:, off : off + fb])

            # ---- s1 = x + y (mixed-dtype TT: bf16 + int8 -> bf16, 1x)
            s1m1 = epool.tile([ROWS, FB], BF16, tag="s1m1")
            nc.vector.tensor_tensor(
                s1m1[:, 0:fb], xt[0:ROWS, 0:fb], yt[:, 0:fb], OP.add
            )

            # ---- |x+y-1| with free accumulate -> sum|x+y-1| (ACT, bias=-1)
            absz = epool.tile([ROWS, FB], BF16, tag="absz")
            nc.scalar.activation(
                out=absz[:, 0:fb],
                in_=s1m1[:, 0:fb],
                func=AF.Abs,
                bias=mone[0:ROWS, :],
                accum_out=stats[0:ROWS, 3 * t : 3 * t + 1],
            )

            # ---- bce partial: sum ln|x+y-1| (ACT with free accumulate)
            lnz = epool.tile([ROWS, FB], BF16, tag="lnz")
            nc.scalar.activation(
                out=lnz[:, 0:fb],
                in_=absz[:, 0:fb],
                func=AF.Ln,
                accum_out=stats[0:ROWS, 3 * t + 2 : 3 * t + 3],
            )

            # ---- gram: transpose each 128-col chunk, matmul-accumulate
            j = 0
            while j < nsub:
                gsz = min(4, nsub - j)
                pst = pst_pool.tile([128, 512], BF16, tag="pst")
                for jj in range(gsz):
                    nc.tensor.transpose(
                        out=pst[:, jj * 128 : (jj + 1) * 128],
                        in_=xt[:, (j + jj) * 128 : (j + jj + 1) * 128],
                        identity=ident[:, :],
                    )
                stsb = spool.tile([128, 512], BF16, tag="stsb")
                nc.vector.tensor_copy(
                    out=stsb[:, 0 : gsz * 128], in_=pst[:, 0 : gsz * 128]
                )
                for jj in range(gsz):
                    cof = jj * 128
                    mm_count += 1
                    nc.tensor.matmul(
                        out=g_ps[:, :],
                        lhsT=stsb[:, cof : cof + 128],
                        rhs=stsb[:, cof : cof + 121],
                        start=(mm_count == 1),
                        stop=(mm_count == nmm),
                        skip_group_check=True,
                    )
                j += gsz

        assert mm_count == nmm, mm_count

        # ---- write results out
        g_sb = sing.tile([128, 128], FP32)
        nc.vector.tensor_copy(out=g_sb[:, 0:121], in_=g_ps[:, :])
        nc.vector.memset(g_sb[:, 121:128], 0.0)
        nc.sync.dma_start(out=g_d, in_=g_sb[:, :])
        nc.sync.dma_start(out=st_d, in_=stats[:, :])

    nc.compile()
    _CACHE["nc"] = nc
    return nc


def _pack(core_slice, dtype, ones_rows=False):
    """[20, HWH] -> [120(+8), L] slab: rows (g, c), zero-padded."""
    nr = 128 if ones_rows else ROWS
    out = np.empty((nr, L), dtype=dtype)
    xp = np.zeros((C, NG * L), dtype=dtype)
    xp[:, :HWH] = core_slice
    out[:ROWS] = xp.reshape(C, NG, L).transpose(1, 0, 2).reshape(ROWS, L)
    if ones_rows:
        out[ROWS:] = 1
    return np.ascontiguousarray(out)


def _run(logit, label_lst, trace=False):
    nc = _build()
    X = np.asarray(logit, dtype=np.float32).reshape(B, C, HW)
    Y = np.asarray(label_lst).reshape(B, C, HW).astype(np.int8)
    ident = np.eye(128, dtype=ml_dtypes.bfloat16)

    in_maps = []
    for k in range(NCORES):
        b, half = k // 2, k % 2
        sl = slice(half * HWH, (half + 1) * HWH)
        in_maps.append(
            {
                "x": _pack(X[b, :, sl], np.float32, ones_rows=True),
                "y": _pack(Y[b, :, sl], np.int8),
                "ident": ident,
            }
        )
    return run_bass_kernel_spmd(nc, in_maps, list(range(NCORES)), trace=trace)


def _combine(results, sum_y):
    """Host-side tiny combine of per-core stats."""
    G = np.zeros((B, C, C), dtype=np.float64)
    sum_x = np.zeros((B, C), dtype=np.float64)
    sabs_r = np.zeros((B, C), dtype=np.float64)
    bce_r = np.zeros((B, C), dtype=np.float64)

    for k in range(NCORES):
        b = k // 2
        r = results[k]
        g = r["g_out"].astype(np.float64)
        st = r["st_out"].astype(np.float64)
        for gi in range(NG):
            sl = slice(gi * C, gi * C + C)
            G[b] += g[sl, sl]
            sum_x[b] += g[sl, 120]
        for t in range(NTILES):
            cols = st[:ROWS, 3 * t : 3 * t + 3].reshape(NG, C, 3)
            sabs_r[b] += cols[:, :, 0].sum(axis=0)
            bce_r[b] += cols[:, :, 2].sum(axis=0)

    # |x+y-1| = 2xy - x - y + 1 ; zero-pads count as x=0,y=0 elements.
    n_padded = HW + 2 * NPAD
    num = 0.5 * (sabs_r + sum_x + sum_y - n_padded)
    s = np.einsum("bii->bi", G)              # sum x^2

    # loss1
    numk = num[:, :KNOWN] + SMOOTH
    denk = s[:, :KNOWN] + sum_y[:, :KNOWN] + SMOOTH
    dice = np.mean(1.0 - numk / denk, axis=0)
    bce = -bce_r[:, :KNOWN].sum(axis=0) / (B * HW)
    loss1 = (dice + bce).sum() / KNOWN

    # loss2
    m = sum_x[:, KNOWN:].sum(axis=0) / (B * HW)
    loss2 = np.sum(-np.log(np.clip(m * 50.0, 1e-300, 1.0))) / (C - KNOWN)

    # loss3
    ratio = (G + SMOOTH) / (s[:, :, None] + s[:, None, :] + SMOOTH)
    M = ratio.mean(axis=0)
    loss3 = (M.sum() - np.trace(M)) / (C * (C - 1))

    loss = (loss1 + loss2 + loss3) * 0.1
    f = np.float32
    return f(loss), f(loss1), f(loss2), f(loss3)


def kernel(logit, label_lst, class_lst=None, **_):
    Yl = np.asarray(label_lst, dtype=np.int64).reshape(B, C, HW)
    sum_y = Yl.sum(axis=2).astype(np.float64)
    res = _run(logit, label_lst, trace=bool(os.environ.get("CDICE_TRACE")))
    out = _combine(res.results, sum_y)
    if os.environ.get("CDICE_TRACE"):
        kernel.last_result = res
    return out


# revision 20
# speedup vs baseline: 6.1361x; 1.0316x over previous
"""CDiceLoss Trainium2 kernel.

Shards B*HW over 8 cores (core = one (batch, half-of-HW) slice). The host
packs each core's logit slice into a [120, 21888] f32 "slab" (rows =
(group g, channel c), 6 groups per channel, zero-padded tail — x=0,y=0
pads are neutral in every statistic) and labels into the same layout as
int8. Per core the Tile kernel computes:
  - G     [128,121] PSUM gram: diag 20x20 blocks sum to sum_hw x_i x_j,
          ones-column 120 gives sum_x per row
  - sabs  = sum |x + y - 1|    ( = 2*sum(x*y) - sum x - sum y + n )
  - bce   = sum ln|x + y - 1|  ( = sum y*ln(x) + (1-y)*ln(1-x) )
sum_y is an exact host-side integer count; the host combines the tiny
per-core stats into (loss, loss1, loss2, loss3).
"""

import os
from contextlib import ExitStack

import numpy as np
import ml_dtypes

import concourse.bass as bass
import concourse.bacc as bacc
import concourse.tile as tile
from concourse import mybir
from concourse.bass_utils import run_bass_kernel_spmd

# ---------------- problem geometry (hardcoded) ----------------
B, C, H, W = 4, 20, 512, 512
HW = H * W                  # 262144
KNOWN = 16
SMOOTH = 1.0
NCORES = 8
HWH = HW // 2               # 131072 positions per core
NG = 6                      # channel-groups per slab
L = 21888                   # padded per-row length: 6*21888 = 131328 = HWH+256
NPAD = NG * L - HWH         # 256 zero pads per channel per core
FB = 2048                   # main tile width
NMAIN = 10
TFB = L - NMAIN * FB        # 1408 = 11*128
NTILES = NMAIN + 1
ROWS = NG * C               # 120

FP32 = mybir.dt.float32
BF16 = mybir.dt.bfloat16
I8 = mybir.dt.int8
OP = mybir.AluOpType
AF = mybir.ActivationFunctionType

_CACHE = {}


def _build():
    """Build (and cache) the per-core bass program."""
    if "nc" in _CACHE:
        return _CACHE["nc"]

    nc = bacc.Bacc(
        "TRN2", target_bir_lowering=False, debug=False, num_devices=NCORES
    )

    x_d = nc.dram_tensor("x", [128, L], BF16, kind="ExternalInput").ap()
    y_d = nc.dram_tensor("y", [ROWS, L], I8, kind="ExternalInput").ap()
    id_d = nc.dram_tensor("ident", [128, 128], BF16, kind="ExternalInput").ap()

    g_d = nc.dram_tensor("g_out", [128, 128], FP32, kind="ExternalOutput").ap()
    st_d = nc.dram_tensor("st_out", [128, 3 * NTILES], FP32, kind="ExternalOutput").ap()

    with tile.TileContext(nc) as tc, ExitStack() as ctx:
        sing = ctx.enter_context(tc.tile_pool(name="sing", bufs=1))
        xpool = ctx.enter_context(tc.tile_pool(name="xpool", bufs=3))
        ypool = ctx.enter_context(tc.tile_pool(name="ypool", bufs=3))
        epool = ctx.enter_context(tc.tile_pool(name="epool", bufs=3))
        spool = ctx.enter_context(tc.tile_pool(name="spool", bufs=4))
        pst_pool = ctx.enter_context(tc.tile_pool(name="pst", bufs=4, space="PSUM"))
        gp_pool = ctx.enter_context(tc.tile_pool(name="gp", bufs=1, space="PSUM"))

        ident = sing.tile([128, 128], BF16)
        nc.sync.dma_start(out=ident[:, :], in_=id_d)

        # stats accumulator columns: [sabs | - | bce] per tile
        stats = sing.tile([128, 3 * NTILES], FP32)
        nc.vector.memset(stats[:, :], 0.0)
        mone = sing.tile([128, 1], FP32)
        nc.vector.memset(mone[:, :], -1.0)

        g_ps = gp_pool.tile([128, 121], FP32)

        # Load finite values into all 128x128 PE weight cells, then run a
        # burst of dummy matmuls during the first DMA to trip the PE HAM
        # clock-gate to 8/8 before real grams arrive.
        warm = pst_pool.tile([128, 128], BF16, tag="pst")
        nc.tensor.transpose(out=warm[:, :], in_=ident[:, :], identity=ident[:, :])
        wps = gp_pool.tile([128, 128], FP32)
        for _ in range(48):
            nc.tensor.matmul(
                out=wps[:, :], lhsT=ident[:, :], rhs=ident[:, :],
                start=True, stop=True, skip_group_check=True,
            )

        nmm = L // 128  # 171 total gram matmuls
        mm_count = 0

        for t in range(NTILES):
            fb = FB if t < NMAIN else TFB
            off = t * FB
            nsub = fb // 128

            # ---- X load: [128, fb] bf16 (host-quantized) via HWDGE;
            # rows 120-127 are host-baked ones (the gram's sum column).
            xt = xpool.tile([128, FB], BF16, tag="xt")
            nc.sync.dma_start(out=xt[:, 0:fb], in_=x_d[:, off : off + fb])

            # ---- Y load: [120, fb] raw int8 (HWDGE; half the write bytes)
            yt = ypool.tile([ROWS, FB], I8, tag="yt")
            nc.sync.dma_start(out=yt[:, 0:fb], in_=y_d[:, off : off + fb])

            # ---- s1 = x + y (mixed-dtype TT: bf16 + int8 -> bf16, 1x)
            s1m1 = epool.tile([ROWS, FB], BF16, tag="s1m1")
            nc.vector.tensor_tensor(
                s1m1[:, 0:fb], xt[0:ROWS, 0:fb], yt[:, 0:fb], OP.add
            )

            # ---- |x+y-1| with free accumulate -> sum|x+y-1| (ACT, bias=-1)
            absz = epool.tile([ROWS, FB], BF16, tag="absz")
            nc.scalar.activation(
                out=absz[:, 0:fb],
                in_=s1m1[:, 0:fb],
                func=AF.Abs,
                bias=mone[0:ROWS, :],
                accum_out=stats[0:ROWS, 3 * t : 3 * t + 1],
            )

            # ---- bce partial: sum ln|x+y-1| (ACT with free accumulate)
            lnz = epool.tile([ROWS, FB], BF16, tag="lnz")
            nc.scalar.activation(
                out=lnz[:, 0:fb],
                in_=absz[:, 0:fb],
                func=AF.Ln,
                accum_out=stats[0:ROWS, 3 * t + 2 : 3 * t + 3],
            )

            # ---- gram: transpose each 128-col chunk, matmul-accumulate
            j = 0
            while j < nsub:
                gsz = min(4, nsub - j)
                pst = pst_pool.tile([128, 512], BF16, tag="pst")
                for jj in range(gsz):
                    nc.tensor.transpose(
                        out=pst[:, jj * 128 : (jj + 1) * 128],
                        in_=xt[:, (j + jj) * 128 : (j + jj + 1) * 128],
                        identity=ident[:, :],
                    )
                stsb = spool.tile([128, 512], BF16, tag="stsb")
                nc.vector.tensor_copy(
                    out=stsb[:, 0 : gsz * 128], in_=pst[:, 0 : gsz * 128]
                )
                for jj in range(gsz):
                    cof = jj * 128
                    mm_count += 1
                    nc.tensor.matmul(
                        out=g_ps[:, :],
                        lhsT=stsb[:, cof : cof + 128],
                        rhs=stsb[:, cof : cof + 121],
                        start=(mm_count == 1),
                        stop=(mm_count == nmm),
                        skip_group_check=True,
                    )
                j += gsz

        assert mm_count == nmm, mm_count

        # ---- write results out
        g_sb = sing.tile([128, 128], FP32)
        nc.vector.tensor_copy(out=g_sb[:, 0:121], in_=g_ps[:, :])
        nc.vector.memset(g_sb[:, 121:128], 0.0)
        nc.sync.dma_start(out=g_d, in_=g_sb[:, :])
        nc.sync.dma_start(out=st_d, in_=stats[:, :])

    nc.compile()
    _CACHE["nc"] = nc
    return nc


def _pack(core_slice, dtype, ones_rows=False):
    """[20, HWH] -> [120(+8), L] slab: rows (g, c), zero-padded."""
    nr = 128 if ones_rows else ROWS
    out = np.empty((nr, L), dtype=dtype)
    xp = np.zeros((C, NG * L), dtype=dtype)
    xp[:, :HWH] = core_slice
    out[:ROWS] = xp.reshape(C, NG, L).transpose(1, 0, 2).reshape(ROWS, L)
    if ones_rows:
        out[ROWS:] = 1
    return np.ascontiguousarray(out)


def _run(logit, label_lst, trace=False):
    nc = _build()
    X = np.asarray(logit, dtype=np.float32).reshape(B, C, HW)
    Y = np.asarray(label_lst).reshape(B, C, HW).astype(np.int8)
    ident = np.eye(128, dtype=ml_dtypes.bfloat16)

    in_maps = []
    for k in range(NCORES):
        b, half = k // 2, k % 2
        sl = slice(half * HWH, (half + 1) * HWH)
        in_maps.append(
            {
                "x": _pack(X[b, :, sl], ml_dtypes.bfloat16, ones_rows=True),
                "y": _pack(Y[b, :, sl], np.int8),
                "ident": ident,
            }
        )
    return run_bass_kernel_spmd(nc, in_maps, list(range(NCORES)), trace=trace)


def _combine(results, sum_y):
    """Host-side tiny combine of per-core stats."""
    G = np.zeros((B, C, C), dtype=np.float64)
    sum_x = np.zeros((B, C), dtype=np.float64)
    sabs_r = np.zeros((B, C), dtype=np.float64)
    bce_r = np.zeros((B, C), dtype=np.float64)

    for k in range(NCORES):
        b = k // 2
        r = results[k]
        g = r["g_out"].astype(np.float64)
        st = r["st_out"].astype(np.float64)
        for gi in range(NG):
            sl = slice(gi * C, gi * C + C)
            G[b] += g[sl, sl]
            sum_x[b] += g[sl, 120]
        for t in range(NTILES):
            cols = st[:ROWS, 3 * t : 3 * t + 3].reshape(NG, C, 3)
            sabs_r[b] += cols[:, :, 0].sum(axis=0)
            bce_r[b] += cols[:, :, 2].sum(axis=0)

    # |x+y-1| = 2xy - x - y + 1 ; zero-pads count as x=0,y=0 elements.
    n_padded = HW + 2 * NPAD
    num = 0.5 * (sabs_r + sum_x + sum_y - n_padded)
    s = np.einsum("bii->bi", G)              # sum x^2

    # loss1
    numk = num[:, :KNOWN] + SMOOTH
    denk = s[:, :KNOWN] + sum_y[:, :KNOWN] + SMOOTH
    dice = np.mean(1.0 - numk / denk, axis=0)
    bce = -bce_r[:, :KNOWN].sum(axis=0) / (B * HW)
    loss1 = (dice + bce).sum() / KNOWN

    # loss2
    m = sum_x[:, KNOWN:].sum(axis=0) / (B * HW)
    loss2 = np.sum(-np.log(np.clip(m * 50.0, 1e-300, 1.0))) / (C - KNOWN)

    # loss3
    ratio = (G + SMOOTH) / (s[:, :, None] + s[:, None, :] + SMOOTH)
    M = ratio.mean(axis=0)
    loss3 = (M.sum() - np.trace(M)) / (C * (C - 1))

    loss = (loss1 + loss2 + loss3) * 0.1
    f = np.float32
    return f(loss), f(loss1), f(loss2), f(loss3)


def kernel(logit, label_lst, class_lst=None, **_):
    Yl = np.asarray(label_lst, dtype=np.int64).reshape(B, C, HW)
    sum_y = Yl.sum(axis=2).astype(np.float64)
    res = _run(logit, label_lst, trace=bool(os.environ.get("CDICE_TRACE")))
    out = _combine(res.results, sum_y)
    if os.environ.get("CDICE_TRACE"):
        kernel.last_result = res
    return out
